# revision 6
# baseline (speedup 1.0000x reference)
"""ASFormer layer (banded local attention + conv FFN) on 8 trn2 NeuronCores.

Sharding: sequence-parallel. (batch, seq-chunk) -> core: B=2 x 4 chunks of 512
tokens. Each core computes output tokens [start, start+512) of one batch,
reading a 768-token halo slice of x (banded attention needs +-64 keys, the
depthwise conv another +-1 token).

Layout strategy per core:
  - x token-major -> LN_a (bn_stats) -> PE-transpose -> a^T feature-major
  - QKV projection in feature-major (f32r matmuls, LN gain/bias folded into
    host-prepared weights via an appended ones-row K-tile)
  - attention computed as S^T[key, query] tiles (k-major) so softmax'd P^T
    feeds the ctx matmul directly with no transposes; softmax denominators
    via an all-ones stationary matmul; 1/denom via ACT ln->exp
  - w_o back to token-major, residual, LN_f/LN_c token-major, PE-transpose,
    depthwise conv as shifted DVE ops in feature-major, pwi (f32r) -> gelu
    (exact erf) -> pwo (bf16) -> token-major + residual -> out
"""

import numpy as np
import ml_dtypes

import concourse.bass as bass
import concourse.tile as tile
import concourse.mybir as mybir
from concourse.bass_utils import run_bass_kernel_spmd

F32 = mybir.dt.float32
F32R = mybir.dt.float32r
BF16 = mybir.dt.bfloat16
AF = mybir.ActivationFunctionType
ALU = mybir.AluOpType

B, S, D, H, HD, FF = 2, 2048, 512, 8, 64, 2048
WIN = 64
NCORES = 8
CHUNK = 512          # output tokens per core
EXT = 768            # x slice per core: tokens [start-128, start+640)
NQ = 640             # query grid: tokens [start-64, start+576)
NEG = -1e30
EPS = 1e-5

# window start (in ext coords) for each of the 6 key tiles
_WJ = [0, 64, 192, 320, 448, 512]
# ctx accumulation: for chunk c (queries ext [256c, 256c+256)), list of
# (j, lo, hi) with lo/hi in ext coords = intersection of window j with chunk
_CTX = {
    0: [(0, 0, 256), (1, 64, 256), (2, 192, 256)],
    1: [(1, 256, 320), (2, 256, 448), (3, 320, 512), (4, 448, 512)],
    2: [(3, 512, 576), (4, 512, 704), (5, 512, 768)],
}


def _fix_excess_waits(nc):
    """The pinned walrus rejects >1 sync wait on most instructions (>2 on
    EventSemaphore). Hoist excess waits onto wait-only EventSemaphore insts."""
    for f in nc.m.functions:
        for bb in f.blocks:
            insts = list(bb.instructions)
            out = []
            changed = False
            for inst in insts:
                si = inst.sync_info
                if si is not None and si.on_wait:
                    cap = 2 if isinstance(inst, mybir.InstEventSemaphore) else 1
                    waits = list(si.on_wait)
                    if len(waits) > cap:
                        extra = waits[cap:]
                        inst.sync_info = mybir.SyncInfo(
                            on_wait=waits[:cap], on_update=list(si.on_update))
                        k = 0
                        while extra:
                            chunk, extra = extra[:2], extra[2:]
                            out.append(mybir.InstEventSemaphore(
                                name=f"{inst.name}-waitsplit{k}",
                                engine=inst.engine, ins=[], outs=[],
                                sync_info=mybir.SyncInfo(on_wait=chunk, on_update=[]),
                            ))
                            k += 1
                        changed = True
                out.append(inst)
            if changed:
                bb.instructions[:] = out


def _build_program(cfg):
    """cfg = (apply_gfbf, apply_gcbc, apply_outmask, use_qkv_bias, use_wo_bias,
    use_pwi_bias, use_pwo_bias) booleans."""
    (apply_gfbf, apply_gcbc, apply_outmask,
     use_qkv_bias, use_wo_bias, use_pwi_bias, use_pwo_bias) = cfg
    nc = bass.Bass(target_bir_lowering=False, trn_type="TRN2")

    d_x = nc.dram_tensor("x_ext", [EXT, D], F32, kind="ExternalInput")
    d_wqkv = nc.dram_tensor("wqkv", [D + 1, 3 * D], F32R, kind="ExternalInput")
    d_wo = nc.dram_tensor("wo", [D + 1, D], BF16, kind="ExternalInput")
    d_wpwi = nc.dram_tensor("wpwi", [D + 1, FF], F32R, kind="ExternalInput")
    d_wpwo = nc.dram_tensor("wpwo", [FF + 1, D], BF16, kind="ExternalInput")
    d_convw = nc.dram_tensor("convw", [D, 3], F32, kind="ExternalInput")
    d_masks = nc.dram_tensor("masks", [6 * 128, 256], BF16, kind="ExternalInput")
    d_ident = nc.dram_tensor("ident", [128, 128], F32, kind="ExternalInput")
    d_ones = nc.dram_tensor("ones_r", [1, EXT], F32R, kind="ExternalInput")
    if apply_gfbf:
        d_gf = nc.dram_tensor("gf_b", [128, D], F32, kind="ExternalInput")
        d_bf = nc.dram_tensor("bf_b", [128, D], F32, kind="ExternalInput")
    if apply_gcbc:
        d_gc = nc.dram_tensor("gc_b", [128, D], F32, kind="ExternalInput")
        d_bc = nc.dram_tensor("bc_b", [128, D], F32, kind="ExternalInput")
        d_ppad = nc.dram_tensor("ppad", [NQ, 1], F32, kind="ExternalInput")
    if apply_outmask:
        d_om = nc.dram_tensor("outmask", [CHUNK, 1], F32, kind="ExternalInput")
    d_out = nc.dram_tensor("out", [CHUNK, D], F32, kind="ExternalOutput")

    import contextlib
    with tile.TileContext(nc) as tc, \
         tc.tile_pool(name="cst", bufs=1) as cst, \
         tc.tile_pool(name="pA", bufs=1) as pA:
        with contextlib.ExitStack() as stack:
            # ---- constants ----
            ident = cst.tile([128, 128], F32)
            ones_r = cst.tile([1, EXT], F32R)
            ones_bf64 = cst.tile([128, HD], BF16)
            nc.vector.memset(ones_bf64, 1.0)
            onesrow_bf = cst.tile([1, 128], BF16)
            nc.vector.memset(onesrow_bf, 1.0)
            eps_sb = cst.tile([128, 1], F32)
            nc.vector.memset(eps_sb, EPS)
            convw_sb = [cst.tile([128, 3], F32, tag=f"convw{i}", name=f"convw{i}") for i in range(4)]
            wo_sb = [cst.tile([64, D], BF16, tag=f"wo{h}", name=f"wo{h}") for h in range(H)]
            wo_bias = cst.tile([1, D], BF16)
            if apply_gfbf:
                gf_sb = cst.tile([128, D], F32)
                bf_sb = cst.tile([128, D], F32)
                nc.sync.dma_start(out=gf_sb, in_=d_gf[:, :])
                nc.sync.dma_start(out=bf_sb, in_=d_bf[:, :])
            if apply_gcbc:
                gc_sb = cst.tile([128, D], F32)
                bc_sb = cst.tile([128, D], F32)
                nc.sync.dma_start(out=gc_sb, in_=d_gc[:, :])
                nc.sync.dma_start(out=bc_sb, in_=d_bc[:, :])
                ppad_sb = [cst.tile([128, 1], F32, tag=f"ppad{t}", name=f"ppad{t}") for t in range(5)]
                for t in range(5):
                    nc.sync.dma_start(out=ppad_sb[t], in_=d_ppad[128 * t:128 * t + 128, :])
            if apply_outmask:
                om_sb = [cst.tile([128, 1], F32, tag=f"om{t}", name=f"om{t}") for t in range(4)]
                for t in range(4):
                    nc.sync.dma_start(out=om_sb[t], in_=d_om[128 * t:128 * t + 128, :])

            # ---- long-lived big tensors (span w_o .. end) ----
            x1 = [pA.tile([128, D], F32, tag=f"x1_{t}", name=f"x1_{t}") for t in range(5)]
            out_sb = [pA.tile([128, D], F32, tag=f"out{t}", name=f"out{t}") for t in range(4)]

            # ---- attention-era tensors (span qkv .. w_o) ----
            pB = stack.enter_context(tc.tile_pool(name="pB", bufs=1))
            qk_t = [pB.tile([128, EXT], F32R, tag=f"qk{i}", name=f"qk{i}") for i in range(8)]
            v_sb = [pB.tile([128, D], BF16, tag=f"v{i}", name=f"v{i}") for i in range(6)]
            ctxT = [pB.tile([64, EXT], BF16, tag=f"ctx{h}", name=f"ctx{h}") for h in range(H)]
            x_q = [pB.tile([128, D], F32, tag=f"xq{t}", name=f"xq{t}") for t in range(5)]
            masks_sb = pB.tile([128, 6 * 256], BF16, name="masks_sb")

            # ============ stage 1: LN_a + transpose ============
            with tc.tile_pool(name="pC", bufs=1) as pC, \
                 tc.tile_pool(name="pCw", bufs=2) as pCw:
                wqkv_sb = [pC.tile([128, 3 * D], F32R, tag=f"wqkv{i}", name=f"wqkv{i}") for i in range(4)]
                wqkv_bias = pC.tile([1, 3 * D], F32R)
                aT_all = pC.tile([128, 4 * EXT], F32R, name="aT_all")
                aT = [aT_all[:, EXT * i:EXT * (i + 1)] for i in range(4)]

                nc.sync.dma_start(out=ident, in_=d_ident[:, :])
                with tc.tile_pool(name="psTR", bufs=2, space="PSUM") as psTR:
                    for t in range(6):
                        xt = pCw.tile([128, D], F32, tag="xt")
                        eng = nc.sync if t % 2 == 0 else nc.scalar
                        eng.dma_start(out=xt, in_=d_x[128 * t:128 * t + 128, :])
                        if t == 0:
                            nc.sync.dma_start(out=ones_r, in_=d_ones[:, :])
                        if t == 2:
                            for i in range(4):
                                nc.sync.dma_start(out=wqkv_sb[i], in_=d_wqkv[128 * i:128 * i + 128, :])
                            nc.sync.dma_start(out=wqkv_bias, in_=d_wqkv[D:D + 1, :])
                        st = pCw.tile([128, 6], F32, tag="st")
                        mv = pCw.tile([128, 2], F32, tag="mv")
                        nc.vector.bn_stats(out=st, in_=xt)
                        nc.vector.bn_aggr(out=mv, in_=st)
                        lnv = pCw.tile([128, 1], F32, tag="lnv")
                        rstd = pCw.tile([128, 1], F32, tag="rstd")
                        nc.scalar.activation(out=lnv, in_=mv[:, 1:2], func=AF.Ln,
                                             bias=eps_sb, scale=1.0)
                        nc.scalar.activation(out=rstd, in_=lnv, func=AF.Exp,
                                             bias=0.0, scale=-0.5)
                        ah = pCw.tile([128, D], F32, tag="ah")
                        nc.vector.tensor_scalar(out=ah, in0=xt, scalar1=mv[:, 0:1],
                                                scalar2=rstd, op0=ALU.subtract, op1=ALU.mult)
                        ptr = psTR.tile([128, 512], F32, tag="ptr")
                        for dd in range(4):
                            nc.tensor.matmul(ptr[:, 128 * dd:128 * dd + 128],
                                             ah[:, 128 * dd:128 * dd + 128], ident,
                                             is_transpose=True, start=(dd == 0),
                                             stop=(dd == 3), skip_group_check=True)
                        outv = aT_all.rearrange("p (g c) -> p g c", g=4)[:, :, 128 * t:128 * t + 128]
                        nc.scalar.copy(outv, ptr.rearrange("p (g c) -> p g c", g=4))

                # ============ stage 2+4 interleaved: V, then per f-tile pair qkv + 2 heads ============
                def emit_qkv_ft(ft):
                    for ch in range(2):
                        pq = psQK.tile([128, 384], F32, tag="pqv", name=f"pq_{ft}_{ch}")
                        for kt in range(4):
                            nc.tensor.matmul(pq, wqkv_sb[kt][:, 128 * ft:128 * ft + 128],
                                             aT[kt][:, 384 * ch:384 * ch + 384],
                                             start=(kt == 0),
                                             stop=(kt == 3 and not use_qkv_bias))
                        if use_qkv_bias:
                            nc.tensor.matmul(pq, wqkv_bias[:, 128 * ft:128 * ft + 128],
                                             ones_r[:, 384 * ch:384 * ch + 384],
                                             start=False, stop=True)
                        nc.vector.tensor_copy(qk_t[ft][:, 384 * ch:384 * ch + 384], pq)

                def emit_v():
                    for tt in range(6):
                        pv = psQK.tile([128, D], F32, tag="pqv", name=f"pv_{tt}")
                        for kt in range(4):
                            nc.tensor.matmul(pv, aT[kt][:, 128 * tt:128 * tt + 128],
                                             wqkv_sb[kt][:, 2 * D:3 * D],
                                             start=(kt == 0),
                                             stop=(kt == 3 and not use_qkv_bias))
                        if use_qkv_bias:
                            nc.tensor.matmul(pv, ones_r[:, 128 * tt:128 * tt + 128],
                                             wqkv_bias[:, 2 * D:3 * D], start=False, stop=True)
                        nc.vector.tensor_copy(v_sb[tt], pv)

                def emit_head(h):
                    hp = 64 * (h % 2)
                    pTraw = pD.tile([128, 6 * 256], BF16, tag="pTraw", name=f"pTraw{h}")
                    for jp in range(3):  # pairs (0,1) (2,3) (4,5)
                        pst = psST.tile([128, 512], F32, tag="pst", name=f"pst{h}_{jp}")
                        for jj in range(2):
                            j = 2 * jp + jj
                            nc.tensor.matmul(
                                pst[:, 256 * jj:256 * jj + 256],
                                qk_t[4 + h // 2][hp:hp + 64, 128 * j:128 * j + 128],
                                qk_t[h // 2][hp:hp + 64, _WJ[j]:_WJ[j] + 256],
                                start=(jj == 0), stop=(jj == 1),
                                skip_group_check=True)
                        nc.scalar.activation(out=pTraw[:, 512 * jp:512 * jp + 512],
                                             in_=pst, func=AF.Exp)
                    pT = pD.tile([128, 6 * 256], BF16, tag="pT", name=f"pT{h}")
                    nc.vector.tensor_mul(out=pT, in0=pTraw, in1=masks_sb)
                    tln = pD.tile([64, 768], F32, tag="tln", name=f"tln{h}")
                    trd = pD.tile([64, 768], F32, tag="trd", name=f"trd{h}")
                    pcxs = []
                    for c in range(3):
                        pcx = psCX.tile([64, 256], F32, tag="pcx", name=f"pcx{h}_{c}")
                        pdn = psDN.tile([64, 256], F32, tag="pdn", name=f"pdn{h}_{c}")
                        pcxs.append(pcx)
                        items = _CTX[c]
                        for idx, (j, lo, hi) in enumerate(items):
                            rhs = pT[:, 256 * j + lo - _WJ[j]:256 * j + hi - _WJ[j]]
                            first = idx == 0
                            last = idx == len(items) - 1
                            nc.tensor.matmul(pcx[:, lo - 256 * c:hi - 256 * c],
                                             v_sb[j][:, 64 * h:64 * h + 64], rhs,
                                             start=first, stop=last,
                                             skip_group_check=True)
                            nc.tensor.matmul(pdn[:, lo - 256 * c:hi - 256 * c],
                                             ones_bf64, rhs,
                                             start=first, stop=last,
                                             skip_group_check=True)
                        nc.scalar.activation(out=tln[:, 256 * c:256 * c + 256],
                                             in_=pdn, func=AF.Ln, bias=0.0, scale=1.0)
                    nc.scalar.activation(out=trd, in_=tln, func=AF.Exp,
                                         bias=0.0, scale=-1.0)
                    for c in range(3):
                        nc.vector.scalar_tensor_tensor(
                            out=ctxT[h][:, 256 * c:256 * c + 256],
                            in0=pcxs[c], scalar=1.0, in1=trd[:, 256 * c:256 * c + 256],
                            op0=ALU.mult, op1=ALU.mult)

                # ---- attention-phase + later loads ----
                nc.sync.dma_start(
                    out=masks_sb.rearrange("p (j q) -> p j q", j=6),
                    in_=d_masks.rearrange("(j p) q -> p j q", j=6))
                for h in range(H):
                    nc.sync.dma_start(out=wo_sb[h], in_=d_wo[64 * h:64 * h + 64, :])
                nc.sync.dma_start(out=wo_bias, in_=d_wo[D:D + 1, :])
                for t in range(5):
                    nc.sync.dma_start(out=x_q[t], in_=d_x[64 + 128 * t:192 + 128 * t, :])
                for i in range(4):
                    nc.sync.dma_start(out=convw_sb[i], in_=d_convw[128 * i:128 * i + 128, :])


                # ============ emission: V, then (qkv pair, 2 heads) x4 ============
                with tc.tile_pool(name="psQK", bufs=2, space="PSUM") as psQK, \
                     tc.tile_pool(name="pD", bufs=3) as pD, \
                     tc.tile_pool(name="psST", bufs=1, space="PSUM") as psST, \
                     tc.tile_pool(name="psCX", bufs=2, space="PSUM") as psCX, \
                     tc.tile_pool(name="psDN", bufs=2, space="PSUM") as psDN:
                    emit_v()
                    for pair in range(4):
                        emit_qkv_ft(pair)
                        emit_qkv_ft(4 + pair)
                        emit_head(2 * pair)
                        emit_head(2 * pair + 1)


            # ============ stage 5: w_o + residual ============
            with tc.tile_pool(name="psAT", bufs=3, space="PSUM") as psAT:
                for tt in range(5):
                    pat = psAT.tile([128, D], F32, tag="pat")
                    for h in range(H):
                        nc.tensor.matmul(pat, ctxT[h][:, 64 + 128 * tt:192 + 128 * tt],
                                         wo_sb[h], start=(h == 0),
                                         stop=(h == H - 1 and not use_wo_bias))
                    if use_wo_bias:
                        nc.tensor.matmul(pat, onesrow_bf, wo_bias, start=False, stop=True)
                    nc.vector.scalar_tensor_tensor(out=x1[tt], in0=pat, scalar=1.0,
                                                   in1=x_q[tt], op0=ALU.mult, op1=ALU.add)

        # pools pB/pC/pD exited above via stack; continue in fresh scope
        with tc.tile_pool(name="pE", bufs=1) as pE, \
             tc.tile_pool(name="pEw", bufs=2) as pEw, \
             tc.tile_pool(name="psT2", bufs=2, space="PSUM") as psT2, \
             tc.tile_pool(name="psPI", bufs=4, space="PSUM") as psPI, \
             tc.tile_pool(name="psPO", bufs=2, space="PSUM") as psPO:
            wpwi_sb = [pE.tile([128, FF], F32R, tag=f"wpwi{i}", name=f"wpwi{i}") for i in range(4)]
            wpwi_bias = pE.tile([1, FF], F32R)
            for i in range(4):
                nc.sync.dma_start(out=wpwi_sb[i], in_=d_wpwi[128 * i:128 * i + 128, :])
            nc.sync.dma_start(out=wpwi_bias, in_=d_wpwi[D:D + 1, :])
            wpwo_sb = [pE.tile([128, D], BF16, tag=f"wpwo{i}", name=f"wpwo{i}") for i in range(16)]
            wpwo_bias = pE.tile([1, D], BF16)
            for i in range(16):
                nc.scalar.dma_start(out=wpwo_sb[i], in_=d_wpwo[128 * i:128 * i + 128, :])
            nc.scalar.dma_start(out=wpwo_bias, in_=d_wpwo[FF:FF + 1, :])
            yT_all = pE.tile([128, 4 * NQ], F32, name="yT_all")
            yT = [yT_all[:, NQ * i:NQ * (i + 1)] for i in range(4)]
            convT = [pE.tile([128, CHUNK], F32R, tag=f"cT{i}", name=f"cT{i}") for i in range(4)]
            g_sb = [pE.tile([128, CHUNK], BF16, tag=f"g{i}", name=f"g{i}") for i in range(16)]
            x1s = [pE.tile([128, D], F32, tag=f"x1s{i}", name=f"x1s{i}") for i in range(4)]

            # x1s = x1 shifted by 64 rows (SBUF->SBUF DMA moves across partitions)
            for t4 in range(4):
                nc.sync.dma_start(out=x1s[t4][0:64, :], in_=x1[t4][64:128, :])
                nc.sync.dma_start(out=x1s[t4][64:128, :], in_=x1[t4 + 1][0:64, :])

            # ---- LN_f / LN_c ----
            epsq_sb = pE.tile([128, 1], F32, name="epsq_sb")
            nc.vector.memset(epsq_sb, EPS * EPS)
            for tt in range(5):
                st1 = pEw.tile([128, 6], F32, tag="st1")
                mv1 = pEw.tile([128, 2], F32, tag="mv1")
                nc.vector.bn_stats(out=st1, in_=x1[tt])
                nc.vector.bn_aggr(out=mv1, in_=st1)
                if not apply_gfbf:
                    # LN_c(LN_f(x)) with unit gain / zero bias collapses to a
                    # single normalization: (x - mu) / sqrt(v*(1+eps) + eps^2)
                    l2 = pEw.tile([128, 1], F32, tag="l2")
                    r2 = pEw.tile([128, 1], F32, tag="r2")
                    nc.scalar.activation(out=l2, in_=mv1[:, 1:2], func=AF.Ln,
                                         bias=epsq_sb, scale=1.0 + EPS)
                    nc.scalar.activation(out=r2, in_=l2, func=AF.Exp, bias=0.0, scale=-0.5)
                    n2 = pEw.tile([128, D], F32, tag="n2")
                    nc.vector.tensor_scalar(out=n2, in0=x1[tt], scalar1=mv1[:, 0:1],
                                            scalar2=r2, op0=ALU.subtract, op1=ALU.mult)
                else:
                    l1 = pEw.tile([128, 1], F32, tag="l1")
                    r1 = pEw.tile([128, 1], F32, tag="r1")
                    nc.scalar.activation(out=l1, in_=mv1[:, 1:2], func=AF.Ln,
                                         bias=eps_sb, scale=1.0)
                    nc.scalar.activation(out=r1, in_=l1, func=AF.Exp, bias=0.0, scale=-0.5)
                    n1 = pEw.tile([128, D], F32, tag="n1")
                    nc.vector.tensor_scalar(out=n1, in0=x1[tt], scalar1=mv1[:, 0:1],
                                            scalar2=r1, op0=ALU.subtract, op1=ALU.mult)
                    y1a = pEw.tile([128, D], F32, tag="y1a")
                    nc.vector.tensor_mul(out=y1a, in0=n1, in1=gf_sb)
                    nc.vector.tensor_add(out=n1, in0=y1a, in1=bf_sb)
                    st2 = pEw.tile([128, 6], F32, tag="st2")
                    mv2 = pEw.tile([128, 2], F32, tag="mv2")
                    nc.vector.bn_stats(out=st2, in_=n1)
                    nc.vector.bn_aggr(out=mv2, in_=st2)
                    l2 = pEw.tile([128, 1], F32, tag="l2")
                    r2 = pEw.tile([128, 1], F32, tag="r2")
                    nc.scalar.activation(out=l2, in_=mv2[:, 1:2], func=AF.Ln,
                                         bias=eps_sb, scale=1.0)
                    nc.scalar.activation(out=r2, in_=l2, func=AF.Exp, bias=0.0, scale=-0.5)
                    n2 = pEw.tile([128, D], F32, tag="n2")
                    nc.vector.tensor_scalar(out=n2, in0=n1, scalar1=mv2[:, 0:1],
                                            scalar2=r2, op0=ALU.subtract, op1=ALU.mult)
                if apply_gcbc:
                    y2a = pEw.tile([128, D], F32, tag="y2a")
                    nc.vector.tensor_mul(out=y2a, in0=n2, in1=gc_sb)
                    nc.vector.tensor_add(out=n2, in0=y2a, in1=bc_sb)
                    nc.vector.tensor_scalar_mul(out=n2, in0=n2, scalar1=ppad_sb[tt])
                pt2 = psT2.tile([128, 512], F32, tag="pt2")
                for dd in range(4):
                    nc.tensor.matmul(pt2[:, 128 * dd:128 * dd + 128],
                                     n2[:, 128 * dd:128 * dd + 128], ident,
                                     is_transpose=True, start=(dd == 0),
                                     stop=(dd == 3), skip_group_check=True)
                outv = yT_all.rearrange("p (g c) -> p g c", g=4)[:, :, 128 * tt:128 * tt + 128]
                nc.scalar.copy(outv, pt2.rearrange("p (g c) -> p g c", g=4))

            # ---- depthwise conv (feature-major, shifted adds) ----
            for dd in range(4):
                c1 = pEw.tile([128, CHUNK], F32, tag="c1")
                nc.vector.tensor_scalar_mul(out=c1, in0=yT[dd][:, 65:65 + CHUNK],
                                            scalar1=convw_sb[dd][:, 2:3])
                c2 = pEw.tile([128, CHUNK], F32, tag="c2")
                nc.vector.scalar_tensor_tensor(out=c2, in0=yT[dd][:, 63:63 + CHUNK],
                                               scalar=convw_sb[dd][:, 0:1], in1=c1,
                                               op0=ALU.mult, op1=ALU.add)
                nc.vector.scalar_tensor_tensor(out=convT[dd], in0=yT[dd][:, 64:64 + CHUNK],
                                               scalar=convw_sb[dd][:, 1:2], in1=c2,
                                               op0=ALU.mult, op1=ALU.add)

            # ---- pwi + gelu ----
            for ffi in range(16):
                ppi = psPI.tile([128, CHUNK], F32, tag="ppi")
                for kt in range(4):
                    nc.tensor.matmul(ppi, wpwi_sb[kt][:, 128 * ffi:128 * ffi + 128],
                                     convT[kt], start=(kt == 0),
                                     stop=(kt == 3 and not use_pwi_bias))
                if use_pwi_bias:
                    nc.tensor.matmul(ppi, wpwi_bias[:, 128 * ffi:128 * ffi + 128],
                                     ones_r[:, 0:CHUNK], start=False, stop=True)
                nc.scalar.activation(out=g_sb[ffi], in_=ppi, func=AF.Gelu)

            # ---- pwo + final residual ----
            for t4 in range(4):
                ppo = psPO.tile([128, D], F32, tag="ppo")
                for ffi in range(16):
                    nc.tensor.matmul(ppo, g_sb[ffi][:, 128 * t4:128 * t4 + 128],
                                     wpwo_sb[ffi], start=(ffi == 0),
                                     stop=(ffi == 15 and not use_pwo_bias))
                if use_pwo_bias:
                    nc.tensor.matmul(ppo, onesrow_bf, wpwo_bias, start=False, stop=True)
                nc.vector.scalar_tensor_tensor(out=out_sb[t4], in0=ppo, scalar=1.0,
                                               in1=x1s[t4], op0=ALU.mult, op1=ALU.add)
                if apply_outmask:
                    nc.vector.tensor_scalar_mul(out=out_sb[t4], in0=out_sb[t4],
                                                scalar1=om_sb[t4])
                nc.sync.dma_start(out=d_out[128 * t4:128 * t4 + 128, :], in_=out_sb[t4])

    _fix_excess_waits(nc)
    return nc



F8 = mybir.dt.float8e4
DRMODE = mybir.MatmulPerfMode.DoubleRow
WS = 32.0

def build_fp8_program():
    nc = bass.Bass(target_bir_lowering=False, trn_type="TRN2")

    d_x = nc.dram_tensor("x_ext", [EXT, D], F32, kind="ExternalInput")
    d_wqkv = nc.dram_tensor("wqkv8", [D, 3 * D], F8, kind="ExternalInput")
    d_wo = nc.dram_tensor("wo8", [D, D], F8, kind="ExternalInput")
    d_wpwi = nc.dram_tensor("wpwi8", [D, FF], F8, kind="ExternalInput")
    d_wpwo = nc.dram_tensor("wpwo8", [FF, D], F8, kind="ExternalInput")
    d_convw = nc.dram_tensor("convw32", [D, 3], F32, kind="ExternalInput")
    d_masks = nc.dram_tensor("masks", [6 * 128, 256], BF16, kind="ExternalInput")
    d_ident = nc.dram_tensor("identb", [128, 128], BF16, kind="ExternalInput")
    d_out = nc.dram_tensor("out", [CHUNK, D], F32, kind="ExternalOutput")

    ESC = 1.0 / (8.0 * WS * WS)     # exp scale: 1/sqrt(HD) / (32*32)
    GSC = 1.0 / (WS * WS)           # gelu input scale
    RS1 = 1.0 / (WS * WS)           # w_o residual scale
    RS2 = 1.0 / WS                  # pwo residual scale

    with tile.TileContext(nc) as tc, \
         tc.tile_pool(name="pA", bufs=1) as pA:
        # ---------------- persistent tiles ----------------
        identb = pA.tile([128, 128], BF16)
        onesb = pA.tile([128, 64], BF16)
        nc.vector.memset(onesb, 1.0)
        eps_sb = pA.tile([128, 1], F32)
        nc.vector.memset(eps_sb, EPS)
        epsq_sb = pA.tile([128, 1], F32)
        nc.vector.memset(epsq_sb, EPS * EPS)

        wqkv_all = pA.tile([128, 4 * 3 * D], F8, name="wqkv_all")
        wo_all = pA.tile([128, 4 * D], F8, name="wo_all")
        wpwi_all = pA.tile([128, 4 * FF], F8, name="wpwi_all")
        wpwo_all = pA.tile([128, 16 * D], F8, name="wpwo_all")
        convw_sb = pA.tile([128, 4 * 3], F32, name="convw_sb")
        masks_sb = pA.tile([128, 6 * 256], BF16, name="masks_sb")

        aT_all = pA.tile([128, 4 * EXT], F8, name="aT_all")
        qk_t = [pA.tile([128, EXT], F8, name=f"qk{i}") for i in range(8)]
        v_sb = [pA.tile([128, D], F8, name=f"v{i}") for i in range(6)]
        ctxT_all = pA.tile([128, 4 * EXT], F8, name="ctxT_all")
        x_q = [pA.tile([128, D], F32, name=f"xq{t}") for t in range(4)]
        x_qe = [pA.tile([1, D], F32, name=f"xqe{i}") for i in range(2)]
        x1 = [pA.tile([128, D], F32, name=f"x1_{t}") for t in range(4)]
        x1e = [pA.tile([1, D], F32, name=f"x1e{i}") for i in range(2)]
        yT_all = pA.tile([128, 4 * 514], BF16, name="yT_all")
        convT = pA.tile([128, 4 * CHUNK], F8, name="convT")
        g_all = pA.tile([128, 16 * CHUNK], F8, name="g_all")
        out_sb = [pA.tile([128, D], F32, name=f"out{t}") for t in range(4)]
        mv_all = pA.tile([128, 2 * 6], F32, name="mv_all")
        rstd_a = pA.tile([128, 6], F32, name="rstd_a")
        mvf_all = pA.tile([128, 2 * 4], F32, name="mvf_all")
        rstd_f = pA.tile([128, 4], F32, name="rstd_f")
        mv_e = [pA.tile([1, 2], F32, name=f"mv_e{i}") for i in range(2)]
        rstd_e = [pA.tile([1, 1], F32, name=f"rstd_e{i}") for i in range(2)]

        wqkv4 = wqkv_all.rearrange("p (g c) -> p g c", g=4)
        wo4 = wo_all.rearrange("p (g c) -> p g c", g=4)
        wpwi4 = wpwi_all.rearrange("p (g c) -> p g c", g=4)
        wpwo16 = wpwo_all.rearrange("p (g c) -> p g c", g=16)
        aT4 = aT_all.rearrange("p (g c) -> p g c", g=4)
        ctxT4 = ctxT_all.rearrange("p (g c) -> p g c", g=4)
        convT4 = convT.rearrange("p (g c) -> p g c", g=4)
        g16 = g_all.rearrange("p (g c) -> p g c", g=16)
        yT4 = yT_all.rearrange("p (g c) -> p g c", g=4)
        cw4 = convw_sb.rearrange("p (g c) -> p g c", g=4)

        # ---------------- all input DMAs up front ----------------
        # sync queue gets ident first (first transpose needs it); x chunks
        # follow in phase 1. scalar queue: wo, x_q, edge rows, wpwi/wpwo.
        nc.sync.dma_start(out=identb, in_=d_ident[:, :])
        nc.scalar.dma_start(out=wo_all.rearrange("p (g c) -> p g c", g=4),
                            in_=d_wo.rearrange("(g p) c -> p g c", g=4))
        nc.scalar.dma_start(out=convw_sb.rearrange("p (g c) -> p g c", g=4),
                            in_=d_convw.rearrange("(g p) c -> p g c", g=4))
        for t in range(4):
            nc.scalar.dma_start(out=x_q[t], in_=d_x[128 + 128 * t:256 + 128 * t, :])
        nc.scalar.dma_start(out=x_qe[0], in_=d_x[127:128, :])
        nc.scalar.dma_start(out=x_qe[1], in_=d_x[640:641, :])
        nc.scalar.dma_start(out=wpwi_all.rearrange("p (g c) -> p g c", g=4),
                            in_=d_wpwi.rearrange("(g p) c -> p g c", g=4))
        nc.scalar.dma_start(out=wpwo_all.rearrange("p (g c) -> p g c", g=16),
                            in_=d_wpwo.rearrange("(g p) c -> p g c", g=16))

        # ---------------- phase 1: x load, LN_a, transpose, V ----------------
        with tc.tile_pool(name="pCw", bufs=3) as pCw, \
             tc.tile_pool(name="psTR", bufs=2, space="PSUM") as psTR, \
             tc.tile_pool(name="psV", bufs=2, space="PSUM") as psV:
            xts = [pA.tile([128, D], F32, name=f"xt_{t}") for t in range(6)]
            # x tiles pipelined on alternating queues for fast first-tile
            for t in range(6):
                eng = nc.sync if t % 2 == 0 else nc.scalar
                eng.dma_start(out=xts[t], in_=d_x[128 * t:128 * t + 128, :])
            nc.sync.dma_start(out=wqkv_all.rearrange("p (g c) -> p g c", g=4),
                              in_=d_wqkv.rearrange("(g p) c -> p g c", g=4))
            nc.sync.dma_start(
                out=masks_sb.rearrange("p (j q) -> p j q", j=6),
                in_=d_masks.rearrange("(j p) q -> p j q", j=6))

            def emit_v(tt):
                pv = psV.tile([128, D], F32, tag="pv", name=f"pv{tt}")
                for pj in range(2):
                    nc.tensor.matmul(
                        pv,
                        aT4[:, 2 * pj:2 * pj + 2, 128 * tt:128 * tt + 128],
                        wqkv4[:, 2 * pj:2 * pj + 2, 2 * D:3 * D],
                        start=(pj == 0), stop=(pj == 1), perf_mode=DRMODE)
                nc.vector.tensor_copy(v_sb[tt], pv)

            for t in range(6):
                xt = xts[t]
                st = pCw.tile([128, 6], F32, tag="st")
                nc.vector.bn_stats(out=st, in_=xt)
                nc.vector.bn_aggr(out=mv_all[:, 2 * t:2 * t + 2], in_=st)
                if t % 2 == 1:
                    # batched rstd for tiles t-1, t
                    vsl = mv_all.rearrange("p (t two) -> p two t", two=2)[:, 1:2, t - 1:t + 1]
                    lsl = pCw.tile([128, 2], F32, tag="lv")
                    nc.scalar.activation(out=lsl, in_=vsl, func=AF.Ln,
                                         bias=eps_sb, scale=1.0)
                    nc.scalar.activation(out=rstd_a[:, t - 1:t + 1], in_=lsl,
                                         func=AF.Exp, bias=0.0, scale=-0.5)
                for tt in (t - 1, t) if t % 2 == 1 else ():
                    xtt = xts[tt]
                    ah = pCw.tile([128, D], BF16, tag="ah")
                    nc.gpsimd.tensor_scalar(
                        out=ah, in0=xtt,
                        scalar1=mv_all[:, 2 * tt:2 * tt + 1],
                        scalar2=rstd_a[:, tt:tt + 1],
                        op0=ALU.subtract, op1=ALU.mult)
                    ptr = psTR.tile([128, 1024], BF16, tag="ptr")
                    for dd in range(4):
                        nc.tensor.matmul(ptr[:, 128 * dd:128 * dd + 128],
                                         ah[:, 128 * dd:128 * dd + 128], identb,
                                         is_transpose=True, start=(dd == 0),
                                         stop=(dd == 3), skip_group_check=True)
                    nc.scalar.copy(aT4[:, :, 128 * tt:128 * tt + 128],
                                   ptr[:, 0:512].rearrange("p (g c) -> p g c", g=4))
                    emit_v(tt)

        # ---------------- phase 2+3: QKV f-major + heads ----------------
        with tc.tile_pool(name="psQK", bufs=1, space="PSUM") as psQK, \
             tc.tile_pool(name="psST", bufs=1, space="PSUM") as psST, \
             tc.tile_pool(name="psCX", bufs=1, space="PSUM") as psCX, \
             tc.tile_pool(name="pD", bufs=2) as pD:

            def emit_qk_ft(ft):
                pq = psQK.tile([128, EXT], F32, tag="pq", name=f"pq{ft}")
                for c0, c1 in ((0, 512), (512, 768)):
                    for pj in range(2):
                        nc.tensor.matmul(
                            pq[:, c0:c1],
                            wqkv4[:, 2 * pj:2 * pj + 2, 128 * ft:128 * ft + 128],
                            aT4[:, 2 * pj:2 * pj + 2, c0:c1],
                            start=(pj == 0), stop=(pj == 1),
                            perf_mode=DRMODE, skip_group_check=True)
                nc.scalar.copy(qk_t[ft], pq)

            def emit_head(h):
                hp = 64 * (h % 2)
                qt = qk_t[h // 2]
                kt_ = qk_t[4 + h // 2]
                pTraw = pD.tile([128, 6 * 256], BF16, tag="pTraw", name=f"pTraw{h}")
                pst = psST.tile([128, 6 * 256], F32, tag="pst", name=f"pst{h}")
                for j in range(6):
                    nc.tensor.matmul(
                        pst[:, 256 * j:256 * j + 256],
                        kt_[hp:hp + 64, 128 * j:128 * j + 128],
                        qt[hp:hp + 64, _WJ[j]:_WJ[j] + 256],
                        start=(j % 2 == 0), stop=(j % 2 == 1),
                        skip_group_check=True)
                nc.scalar.activation(out=pTraw, in_=pst, func=AF.Exp,
                                     bias=0.0, scale=ESC)
                pT = pD.tile([128, 6 * 256], BF16, tag="pT", name=f"pT{h}")
                nc.vector.tensor_mul(out=pT, in0=pTraw, in1=masks_sb)
                pcxdn = psCX.tile([128, 1024], F32, tag="pcxdn", name=f"pcxdn{h}")
                for c in range(3):
                    items = _CTX[c]
                    for idx, (j, lo, hi) in enumerate(items):
                        rhs = pT[:, 256 * j + lo - _WJ[j]:256 * j + hi - _WJ[j]]
                        first = idx == 0 and c in (0, 2)
                        last = idx == len(items) - 1
                        nc.tensor.matmul(pcxdn[0:64, lo:hi],
                                         v_sb[j][:, 64 * h:64 * h + 64], rhs,
                                         start=first, stop=last,
                                         skip_group_check=True)
                        nc.tensor.matmul(pcxdn[64:128, lo:hi],
                                         onesb, rhs,
                                         start=first, stop=last,
                                         skip_group_check=True)
                trd = pD.tile([64, EXT], F32, tag="trd", name=f"trd{h}")
                if h % 2 == 0:
                    nc.vector.reciprocal(out=trd, in_=pcxdn[64:128, 0:EXT])
                else:
                    tln = pD.tile([64, EXT], F32, tag="tln", name=f"tln{h}")
                    nc.scalar.activation(out=tln, in_=pcxdn[64:128, 0:EXT],
                                         func=AF.Ln, bias=0.0, scale=1.0)
                    nc.scalar.activation(out=trd, in_=tln, func=AF.Exp,
                                         bias=0.0, scale=-1.0)
                nc.vector.tensor_tensor(
                    out=ctxT4[hp:hp + 64, h // 2, :],
                    in0=pcxdn[0:64, 0:EXT], in1=trd, op=ALU.mult)

            emit_qk_ft(0)
            emit_qk_ft(4)
            for pair in range(4):
                emit_head(2 * pair)
                if pair < 3:
                    emit_qk_ft(pair + 1)
                    emit_qk_ft(pair + 5)
                emit_head(2 * pair + 1)

        # ---------------- phase 4: w_o + residual + LN_f + transpose ----------
        with tc.tile_pool(name="pEw", bufs=3) as pEw, \
             tc.tile_pool(name="psAT", bufs=2, space="PSUM") as psAT, \
             tc.tile_pool(name="psAE", bufs=1, space="PSUM") as psAE, \
             tc.tile_pool(name="psT2", bufs=2, space="PSUM") as psT2:
            for tt in range(4):
                pat = psAT.tile([128, D], F32, tag="pat", name=f"pat{tt}")
                for g in range(2):
                    nc.tensor.matmul(
                        pat,
                        ctxT4[:, 2 * g:2 * g + 2, 128 + 128 * tt:256 + 128 * tt],
                        wo4[:, 2 * g:2 * g + 2, :],
                        start=(g == 0), stop=(g == 1), perf_mode=DRMODE)
                nc.vector.scalar_tensor_tensor(
                    out=x1[tt], in0=pat, scalar=RS1, in1=x_q[tt],
                    op0=ALU.mult, op1=ALU.add)
                stf = pEw.tile([128, 6], F32, tag="stf")
                nc.vector.bn_stats(out=stf, in_=x1[tt])
                nc.vector.bn_aggr(out=mvf_all[:, 2 * tt:2 * tt + 2], in_=stf)
            # edge rows (ext 127 and 640) -> two [1,512] psum tiles
            pes = []
            for ei, col in enumerate((127, 640)):
                pe_ = psAE.tile([1, D], F32, tag=f"pate{ei}", name=f"pate{ei}")
                for g in range(2):
                    nc.tensor.matmul(
                        pe_,
                        ctxT4[:, 2 * g:2 * g + 2, col:col + 1],
                        wo4[:, 2 * g:2 * g + 2, :],
                        start=(g == 0), stop=(g == 1), perf_mode=DRMODE)
                pes.append(pe_)
            for ei in range(2):
                nc.vector.scalar_tensor_tensor(
                    out=x1e[ei], in0=pes[ei], scalar=RS1,
                    in1=x_qe[ei], op0=ALU.mult, op1=ALU.add)
                ste = pEw.tile([1, 6], F32, tag=f"ste{ei}")
                nc.vector.bn_stats(out=ste, in_=x1e[ei])
                nc.vector.bn_aggr(out=mv_e[ei], in_=ste)

            # batched LN_f scalars (collapsed double-LN)
            vslf = mvf_all.rearrange("p (t two) -> p two t", two=2)[:, 1:2, :]
            lf = pEw.tile([128, 4], F32, tag="lf")
            nc.scalar.activation(out=lf, in_=vslf, func=AF.Ln,
                                 bias=epsq_sb, scale=1.0 + EPS)
            nc.scalar.activation(out=rstd_f, in_=lf, func=AF.Exp,
                                 bias=0.0, scale=-0.5)
            for ei in range(2):
                le = pEw.tile([1, 1], F32, tag=f"le{ei}")
                nc.scalar.activation(out=le, in_=mv_e[ei][:, 1:2], func=AF.Ln,
                                     bias=epsq_sb[0:1, :], scale=1.0 + EPS)
                nc.scalar.activation(out=rstd_e[ei], in_=le, func=AF.Exp,
                                     bias=0.0, scale=-0.5)

            for tt in range(4):
                n2 = pEw.tile([128, D], BF16, tag="n2")
                nc.gpsimd.tensor_scalar(
                    out=n2, in0=x1[tt],
                    scalar1=mvf_all[:, 2 * tt:2 * tt + 1],
                    scalar2=rstd_f[:, tt:tt + 1],
                    op0=ALU.subtract, op1=ALU.mult)
                pt2 = psT2.tile([128, 1024], BF16, tag="pt2")
                for dd in range(4):
                    nc.tensor.matmul(pt2[:, 128 * dd:128 * dd + 128],
                                     n2[:, 128 * dd:128 * dd + 128], identb,
                                     is_transpose=True, start=(dd == 0),
                                     stop=(dd == 3), skip_group_check=True)
                nc.scalar.copy(yT4[:, :, 1 + 128 * tt:129 + 128 * tt],
                               pt2[:, 0:512].rearrange("p (g c) -> p g c", g=4))
            # edge LN + transpose -> yT cols 0 and 513
            pt2e = psAE.tile([128, 1024], BF16, tag="pt2e", name="pt2e")
            for ei in range(2):
                n2e = pEw.tile([1, D], BF16, tag=f"n2e{ei}")
                nc.gpsimd.tensor_scalar(
                    out=n2e, in0=x1e[ei], scalar1=mv_e[ei][:, 0:1],
                    scalar2=rstd_e[ei], op0=ALU.subtract, op1=ALU.mult)
                for dd in range(4):
                    k = 2 * (4 * ei + dd)
                    nc.tensor.matmul(pt2e[:, k:k + 1],
                                     n2e[:, 128 * dd:128 * dd + 128],
                                     identb[0:1, 0:1],
                                     is_transpose=True,
                                     start=(ei == 0 and dd == 0),
                                     stop=(ei == 1 and dd == 3),
                                     skip_group_check=True)
            ecol = (0, 513)
            for ei in range(2):
                for dd in range(4):
                    k = 2 * (4 * ei + dd)
                    nc.scalar.copy(yT4[:, dd, ecol[ei]:ecol[ei] + 1],
                                   pt2e[:, k:k + 1])

        # ---------------- phase 5: conv ----------------
        with tc.tile_pool(name="pF", bufs=2) as pF:
            for dd in range(4):
                c1 = pF.tile([128, CHUNK], BF16, tag="c1")
                nc.vector.tensor_scalar_mul(out=c1, in0=yT4[:, dd, 2:514],
                                            scalar1=cw4[:, dd, 2:3])
                c2 = pF.tile([128, CHUNK], BF16, tag="c2")
                nc.vector.scalar_tensor_tensor(
                    out=c2, in0=yT4[:, dd, 0:512], scalar=cw4[:, dd, 0:1],
                    in1=c1, op0=ALU.mult, op1=ALU.add)
                nc.vector.scalar_tensor_tensor(
                    out=convT4[:, dd, :], in0=yT4[:, dd, 1:513],
                    scalar=cw4[:, dd, 1:2], in1=c2, op0=ALU.mult, op1=ALU.add)

        # ---------------- phase 6: pwi + gelu ----------------
        with tc.tile_pool(name="psPI", bufs=2, space="PSUM") as psPI, \
             tc.tile_pool(name="psPO", bufs=1, space="PSUM") as psPO:
            ppos = [psPO.tile([128, D], F32, tag=f"ppo{t4}", name=f"ppo{t4}")
                    for t4 in range(4)]
            for j in range(8):
                ppi = psPI.tile([128, 2 * CHUNK], F32, tag="ppi", name=f"ppi{j}")
                for sub in range(2):
                    ffi = 2 * j + sub
                    for pj in range(2):
                        nc.tensor.matmul(
                            ppi[:, 512 * sub:512 * sub + 512],
                            wpwi4[:, 2 * pj:2 * pj + 2, 128 * ffi:128 * ffi + 128],
                            convT4[:, 2 * pj:2 * pj + 2, :],
                            start=(pj == 0), stop=(pj == 1),
                            perf_mode=DRMODE, skip_group_check=True)
                nc.scalar.activation(
                    out=g16[:, 2 * j:2 * j + 2, :], in_=ppi,
                    func=AF.Gelu, bias=0.0, scale=GSC)
                for t4 in range(4):
                    nc.tensor.matmul(
                        ppos[t4],
                        g16[:, 2 * j:2 * j + 2, 128 * t4:128 * t4 + 128],
                        wpwo16[:, 2 * j:2 * j + 2, :],
                        start=(j == 0), stop=(j == 7), perf_mode=DRMODE,
                        skip_group_check=True)
            for t4 in range(4):
                nc.vector.scalar_tensor_tensor(
                    out=out_sb[t4], in0=ppos[t4], scalar=RS2, in1=x1[t4],
                    op0=ALU.mult, op1=ALU.add)
                nc.sync.dma_start(out=d_out[128 * t4:128 * t4 + 128, :],
                                  in_=out_sb[t4])

    return nc


_PROG_CACHE = {}


def _get_program(cfg):
    if cfg not in _PROG_CACHE:
        if cfg == "fp8":
            nc = build_fp8_program()
            _fix_excess_waits(nc)
            _PROG_CACHE[cfg] = nc
        else:
            _PROG_CACHE[cfg] = _build_program(cfg)
    return _PROG_CACHE[cfg]


def _build_masks(key_mask_row, start):
    """Multiplicative {0,1} masks [6*128, 256] bf16 for one core (k-major S^T)."""
    out = np.zeros((6, 128, 256), np.float32)
    # key usability per ext position
    g_all = start - 128 + np.arange(EXT)
    k_ok = (g_all >= 0) & (g_all < S)
    k_ok &= key_mask_row[np.clip(g_all, 0, S - 1)]
    # a query is "live" if it is a real query position AND has >=1 usable
    # in-band key; otherwise it self-attends (finite junk, later zeroed --
    # matches the reference, whose all-masked rows are zeroed by the final
    # mask multiply before anything can observe them)
    q_live = np.zeros(EXT, bool)
    for e_q in range(64, 704):
        g_q = start - 128 + e_q
        if 0 <= g_q < S:
            lo, hi = max(0, e_q - WIN), min(EXT, e_q + WIN + 1)
            q_live[e_q] = k_ok[lo:hi].any()
    for j in range(6):
        kl = np.arange(128)
        ql = np.arange(256)
        e_k = 128 * j + kl[:, None]           # [128, 1]
        e_q = _WJ[j] + ql[None, :]            # [1, 256]
        band = np.abs(e_q - e_k) <= WIN
        ok = (q_live[e_q] & k_ok[e_k] & band) | ((~q_live[e_q]) & (e_k == e_q))
        out[j][np.broadcast_to(ok, (128, 256))] = 1.0
    return np.ascontiguousarray(out.reshape(6 * 128, 256).astype(ml_dtypes.bfloat16))


def prepare(**inputs):
    x = np.ascontiguousarray(np.asarray(inputs["x"], np.float32))
    key_mask = np.asarray(inputs["mask"]).astype(bool)
    ln_a_g = np.asarray(inputs["ln_a_g"], np.float32)
    ln_a_b = np.asarray(inputs["ln_a_b"], np.float32)
    w_qkv = np.asarray(inputs["w_qkv"], np.float32)
    b_qkv = np.asarray(inputs["b_qkv"], np.float32)
    w_o = np.asarray(inputs["w_o"], np.float32)
    b_o = np.asarray(inputs["b_o"], np.float32)
    ln_f_g = np.asarray(inputs["ln_f_g"], np.float32)
    ln_f_b = np.asarray(inputs["ln_f_b"], np.float32)
    ln_c_g = np.asarray(inputs["ln_c_g"], np.float32)
    ln_c_b = np.asarray(inputs["ln_c_b"], np.float32)
    dw_w = np.asarray(inputs["dw_w"], np.float32)
    dw_b = np.asarray(inputs["dw_b"], np.float32)
    pwi_w = np.asarray(inputs["pwi_w"], np.float32)
    pwi_b = np.asarray(inputs["pwi_b"], np.float32)
    pwo_w = np.asarray(inputs["pwo_w"], np.float32)
    pwo_b = np.asarray(inputs["pwo_b"], np.float32)

    apply_gfbf = not (np.all(ln_f_g == 1.0) and np.all(ln_f_b == 0.0))
    apply_gcbc = not np.all(ln_c_b == 0.0)
    apply_outmask = not key_mask.all()

    # ---- host weight prep ----
    scale = np.float32(1.0 / np.sqrt(HD))
    Wt = w_qkv.T.astype(np.float64)                      # [D, 3D]
    Wg = Wt * ln_a_g[:, None].astype(np.float64)
    brow = ln_a_b.astype(np.float64) @ Wt + b_qkv
    Wg[:, :D] *= scale
    brow[:D] *= scale
    wqkv_hat = np.ascontiguousarray(
        np.vstack([Wg, brow[None, :]]).astype(np.float32))

    wo_hat = np.ascontiguousarray(
        np.vstack([w_o.T, b_o[None, :]]).astype(ml_dtypes.bfloat16))

    if not apply_gcbc:
        convw_eff = dw_w[:, 0, :] * ln_c_g[:, None]
        cc = dw_b
    else:
        convw_eff = dw_w[:, 0, :]
        cc = dw_b
    convw_eff = np.ascontiguousarray(convw_eff.astype(np.float32))

    pwi_bias_row = pwi_b + pwi_w @ cc
    wpwi_hat = np.ascontiguousarray(
        np.vstack([pwi_w.T, pwi_bias_row[None, :]]).astype(np.float32))
    wpwo_hat = np.ascontiguousarray(
        np.vstack([pwo_w.T, pwo_b[None, :]]).astype(ml_dtypes.bfloat16))
    use_qkv_bias = bool(np.any(brow != 0.0))
    use_wo_bias = bool(np.any(b_o != 0.0))
    use_pwi_bias = bool(np.any(pwi_bias_row != 0.0))
    use_pwo_bias = bool(np.any(pwo_b != 0.0))
    cfg = (apply_gfbf, apply_gcbc, apply_outmask,
           use_qkv_bias, use_wo_bias, use_pwi_bias, use_pwo_bias)

    ident = np.eye(128, dtype=np.float32)
    ones_r = np.ones((1, EXT), np.float32)

    fast = not any(cfg)
    if fast:
        E4 = ml_dtypes.float8_e4m3
        wqkv8 = np.ascontiguousarray(
            (w_qkv.T * ln_a_g[:, None] * WS).astype(E4))
        wo8 = np.ascontiguousarray((w_o.T * WS).astype(E4))
        wpwi8 = np.ascontiguousarray((pwi_w.T * WS).astype(E4))
        wpwo8 = np.ascontiguousarray((pwo_w.T * WS).astype(E4))
        convw32 = np.ascontiguousarray(
            (dw_w[:, 0, :] * ln_c_g[:, None] * WS).astype(np.float32))
        identb = np.eye(128, dtype=ml_dtypes.bfloat16)
        nc = _get_program("fp8")
        in_maps = []
        for core in range(NCORES):
            b, c = divmod(core, 4)
            start = CHUNK * c
            lo, hi = start - 128, start + 640
            x_ext = np.zeros((EXT, D), np.float32)
            s0, s1 = max(lo, 0), min(hi, S)
            x_ext[s0 - lo:s1 - lo] = x[b, s0:s1]
            in_maps.append({
                "x_ext": x_ext,
                "wqkv8": wqkv8,
                "wo8": wo8,
                "wpwi8": wpwi8,
                "wpwo8": wpwo8,
                "convw32": convw32,
                "masks": _build_masks(key_mask[b], start),
                "identb": identb,
            })

        def assemble(per_core_outs):
            out = np.empty((B, S, D), np.float32)
            for core in range(NCORES):
                b, c = divmod(core, 4)
                out[b, CHUNK * c:CHUNK * (c + 1)] = per_core_outs[core]
            return out

        return nc, in_maps, assemble

    nc = _get_program(cfg)

    in_maps = []
    for core in range(NCORES):
        b, c = divmod(core, 4)
        start = CHUNK * c
        lo, hi = start - 128, start + 640
        x_ext = np.zeros((EXT, D), np.float32)
        s0, s1 = max(lo, 0), min(hi, S)
        x_ext[s0 - lo:s1 - lo] = x[b, s0:s1]
        m = {
            "x_ext": x_ext,
            "wqkv": wqkv_hat,
            "wo": wo_hat,
            "wpwi": wpwi_hat,
            "wpwo": wpwo_hat,
            "convw": convw_eff,
            "masks": _build_masks(key_mask[b], start),
            "ident": ident,
            "ones_r": ones_r,
        }
        if apply_gfbf:
            m["gf_b"] = np.ascontiguousarray(
                np.broadcast_to(ln_f_g[None, :], (128, D)).astype(np.float32))
            m["bf_b"] = np.ascontiguousarray(
                np.broadcast_to(ln_f_b[None, :], (128, D)).astype(np.float32))
        if apply_gcbc:
            m["gc_b"] = np.ascontiguousarray(
                np.broadcast_to(ln_c_g[None, :], (128, D)).astype(np.float32))
            m["bc_b"] = np.ascontiguousarray(
                np.broadcast_to(ln_c_b[None, :], (128, D)).astype(np.float32))
            gq = start - 64 + np.arange(NQ)
            m["ppad"] = ((gq >= 0) & (gq < S)).astype(np.float32)[:, None]
        if apply_outmask:
            m["outmask"] = key_mask[b, start:start + CHUNK].astype(np.float32)[:, None]
        in_maps.append(m)

    def assemble(per_core_outs):
        out = np.empty((B, S, D), np.float32)
        for core in range(NCORES):
            b, c = divmod(core, 4)
            out[b, CHUNK * c:CHUNK * (c + 1)] = per_core_outs[core]
        return out

    return nc, in_maps, assemble


def kernel(**inputs):
    nc, in_maps, assemble = prepare(**inputs)
    res = run_bass_kernel_spmd(nc, in_maps, core_ids=list(range(NCORES)))
    return assemble([res.results[core]["out"] for core in range(NCORES)])



# revision 8
# speedup vs baseline: 1.1559x; 1.1559x over previous
"""ASFormer layer (banded local attention + conv FFN) on 8 trn2 NeuronCores.

Sharding: sequence-parallel. (batch, seq-chunk) -> core: B=2 x 4 chunks of 512
tokens. Each core computes output tokens [start, start+512) of one batch,
reading a 768-token halo slice of x (banded attention needs +-64 keys, the
depthwise conv another +-1 token).

Layout strategy per core:
  - x token-major -> LN_a (bn_stats) -> PE-transpose -> a^T feature-major
  - QKV projection in feature-major (f32r matmuls, LN gain/bias folded into
    host-prepared weights via an appended ones-row K-tile)
  - attention computed as S^T[key, query] tiles (k-major) so softmax'd P^T
    feeds the ctx matmul directly with no transposes; softmax denominators
    via an all-ones stationary matmul; 1/denom via ACT ln->exp
  - w_o back to token-major, residual, LN_f/LN_c token-major, PE-transpose,
    depthwise conv as shifted DVE ops in feature-major, pwi (f32r) -> gelu
    (exact erf) -> pwo (bf16) -> token-major + residual -> out
"""

import numpy as np
import ml_dtypes

import concourse.bass as bass
import concourse.tile as tile
import concourse.mybir as mybir
from concourse.bass_utils import run_bass_kernel_spmd

F32 = mybir.dt.float32
F32R = mybir.dt.float32r
BF16 = mybir.dt.bfloat16
AF = mybir.ActivationFunctionType
ALU = mybir.AluOpType

B, S, D, H, HD, FF = 2, 2048, 512, 8, 64, 2048
WIN = 64
NCORES = 8
CHUNK = 512          # output tokens per core
EXT = 768            # x slice per core: tokens [start-128, start+640)
NQ = 640             # query grid: tokens [start-64, start+576)
NEG = -1e30
EPS = 1e-5

# window start (in ext coords) for each of the 6 key tiles
_WJ = [0, 64, 192, 320, 448, 512]
# ctx accumulation: for chunk c (queries ext [256c, 256c+256)), list of
# (j, lo, hi) with lo/hi in ext coords = intersection of window j with chunk
_CTX = {
    0: [(0, 0, 256), (1, 64, 256), (2, 192, 256)],
    1: [(1, 256, 320), (2, 256, 448), (3, 320, 512), (4, 448, 512)],
    2: [(3, 512, 576), (4, 512, 704), (5, 512, 768)],
}


def _fix_excess_waits(nc):
    """The pinned walrus rejects >1 sync wait on most instructions (>2 on
    EventSemaphore). Hoist excess waits onto wait-only EventSemaphore insts."""
    for f in nc.m.functions:
        for bb in f.blocks:
            insts = list(bb.instructions)
            out = []
            changed = False
            for inst in insts:
                si = inst.sync_info
                if si is not None and si.on_wait:
                    cap = 2 if isinstance(inst, mybir.InstEventSemaphore) else 1
                    waits = list(si.on_wait)
                    if len(waits) > cap:
                        extra = waits[cap:]
                        inst.sync_info = mybir.SyncInfo(
                            on_wait=waits[:cap], on_update=list(si.on_update))
                        k = 0
                        while extra:
                            chunk, extra = extra[:2], extra[2:]
                            out.append(mybir.InstEventSemaphore(
                                name=f"{inst.name}-waitsplit{k}",
                                engine=inst.engine, ins=[], outs=[],
                                sync_info=mybir.SyncInfo(on_wait=chunk, on_update=[]),
                            ))
                            k += 1
                        changed = True
                out.append(inst)
            if changed:
                bb.instructions[:] = out


def _build_program(cfg):
    """cfg = (apply_gfbf, apply_gcbc, apply_outmask, use_qkv_bias, use_wo_bias,
    use_pwi_bias, use_pwo_bias) booleans."""
    (apply_gfbf, apply_gcbc, apply_outmask,
     use_qkv_bias, use_wo_bias, use_pwi_bias, use_pwo_bias) = cfg
    nc = bass.Bass(target_bir_lowering=False, trn_type="TRN2")

    d_x = nc.dram_tensor("x_ext", [EXT, D], F32, kind="ExternalInput")
    d_wqkv = nc.dram_tensor("wqkv", [D + 1, 3 * D], F32R, kind="ExternalInput")
    d_wo = nc.dram_tensor("wo", [D + 1, D], BF16, kind="ExternalInput")
    d_wpwi = nc.dram_tensor("wpwi", [D + 1, FF], F32R, kind="ExternalInput")
    d_wpwo = nc.dram_tensor("wpwo", [FF + 1, D], BF16, kind="ExternalInput")
    d_convw = nc.dram_tensor("convw", [D, 3], F32, kind="ExternalInput")
    d_masks = nc.dram_tensor("masks", [6 * 128, 256], BF16, kind="ExternalInput")
    d_ident = nc.dram_tensor("ident", [128, 128], F32, kind="ExternalInput")
    d_ones = nc.dram_tensor("ones_r", [1, EXT], F32R, kind="ExternalInput")
    if apply_gfbf:
        d_gf = nc.dram_tensor("gf_b", [128, D], F32, kind="ExternalInput")
        d_bf = nc.dram_tensor("bf_b", [128, D], F32, kind="ExternalInput")
    if apply_gcbc:
        d_gc = nc.dram_tensor("gc_b", [128, D], F32, kind="ExternalInput")
        d_bc = nc.dram_tensor("bc_b", [128, D], F32, kind="ExternalInput")
        d_ppad = nc.dram_tensor("ppad", [NQ, 1], F32, kind="ExternalInput")
    if apply_outmask:
        d_om = nc.dram_tensor("outmask", [CHUNK, 1], F32, kind="ExternalInput")
    d_out = nc.dram_tensor("out", [CHUNK, D], F32, kind="ExternalOutput")

    import contextlib
    with tile.TileContext(nc) as tc, \
         tc.tile_pool(name="cst", bufs=1) as cst, \
         tc.tile_pool(name="pA", bufs=1) as pA:
        with contextlib.ExitStack() as stack:
            # ---- constants ----
            ident = cst.tile([128, 128], F32)
            ones_r = cst.tile([1, EXT], F32R)
            ones_bf64 = cst.tile([128, HD], BF16)
            nc.vector.memset(ones_bf64, 1.0)
            onesrow_bf = cst.tile([1, 128], BF16)
            nc.vector.memset(onesrow_bf, 1.0)
            eps_sb = cst.tile([128, 1], F32)
            nc.vector.memset(eps_sb, EPS)
            convw_sb = [cst.tile([128, 3], F32, tag=f"convw{i}", name=f"convw{i}") for i in range(4)]
            wo_sb = [cst.tile([64, D], BF16, tag=f"wo{h}", name=f"wo{h}") for h in range(H)]
            wo_bias = cst.tile([1, D], BF16)
            if apply_gfbf:
                gf_sb = cst.tile([128, D], F32)
                bf_sb = cst.tile([128, D], F32)
                nc.sync.dma_start(out=gf_sb, in_=d_gf[:, :])
                nc.sync.dma_start(out=bf_sb, in_=d_bf[:, :])
            if apply_gcbc:
                gc_sb = cst.tile([128, D], F32)
                bc_sb = cst.tile([128, D], F32)
                nc.sync.dma_start(out=gc_sb, in_=d_gc[:, :])
                nc.sync.dma_start(out=bc_sb, in_=d_bc[:, :])
                ppad_sb = [cst.tile([128, 1], F32, tag=f"ppad{t}", name=f"ppad{t}") for t in range(5)]
                for t in range(5):
                    nc.sync.dma_start(out=ppad_sb[t], in_=d_ppad[128 * t:128 * t + 128, :])
            if apply_outmask:
                om_sb = [cst.tile([128, 1], F32, tag=f"om{t}", name=f"om{t}") for t in range(4)]
                for t in range(4):
                    nc.sync.dma_start(out=om_sb[t], in_=d_om[128 * t:128 * t + 128, :])

            # ---- long-lived big tensors (span w_o .. end) ----
            x1 = [pA.tile([128, D], F32, tag=f"x1_{t}", name=f"x1_{t}") for t in range(5)]
            out_sb = [pA.tile([128, D], F32, tag=f"out{t}", name=f"out{t}") for t in range(4)]

            # ---- attention-era tensors (span qkv .. w_o) ----
            pB = stack.enter_context(tc.tile_pool(name="pB", bufs=1))
            qk_t = [pB.tile([128, EXT], F32R, tag=f"qk{i}", name=f"qk{i}") for i in range(8)]
            v_sb = [pB.tile([128, D], BF16, tag=f"v{i}", name=f"v{i}") for i in range(6)]
            ctxT = [pB.tile([64, EXT], BF16, tag=f"ctx{h}", name=f"ctx{h}") for h in range(H)]
            x_q = [pB.tile([128, D], F32, tag=f"xq{t}", name=f"xq{t}") for t in range(5)]
            masks_sb = pB.tile([128, 6 * 256], BF16, name="masks_sb")

            # ============ stage 1: LN_a + transpose ============
            with tc.tile_pool(name="pC", bufs=1) as pC, \
                 tc.tile_pool(name="pCw", bufs=2) as pCw:
                wqkv_sb = [pC.tile([128, 3 * D], F32R, tag=f"wqkv{i}", name=f"wqkv{i}") for i in range(4)]
                wqkv_bias = pC.tile([1, 3 * D], F32R)
                aT_all = pC.tile([128, 4 * EXT], F32R, name="aT_all")
                aT = [aT_all[:, EXT * i:EXT * (i + 1)] for i in range(4)]

                nc.sync.dma_start(out=ident, in_=d_ident[:, :])
                with tc.tile_pool(name="psTR", bufs=2, space="PSUM") as psTR:
                    for t in range(6):
                        xt = pCw.tile([128, D], F32, tag="xt")
                        eng = nc.sync if t % 2 == 0 else nc.scalar
                        eng.dma_start(out=xt, in_=d_x[128 * t:128 * t + 128, :])
                        if t == 0:
                            nc.sync.dma_start(out=ones_r, in_=d_ones[:, :])
                        if t == 2:
                            for i in range(4):
                                nc.sync.dma_start(out=wqkv_sb[i], in_=d_wqkv[128 * i:128 * i + 128, :])
                            nc.sync.dma_start(out=wqkv_bias, in_=d_wqkv[D:D + 1, :])
                        st = pCw.tile([128, 6], F32, tag="st")
                        mv = pCw.tile([128, 2], F32, tag="mv")
                        nc.vector.bn_stats(out=st, in_=xt)
                        nc.vector.bn_aggr(out=mv, in_=st)
                        lnv = pCw.tile([128, 1], F32, tag="lnv")
                        rstd = pCw.tile([128, 1], F32, tag="rstd")
                        nc.scalar.activation(out=lnv, in_=mv[:, 1:2], func=AF.Ln,
                                             bias=eps_sb, scale=1.0)
                        nc.scalar.activation(out=rstd, in_=lnv, func=AF.Exp,
                                             bias=0.0, scale=-0.5)
                        ah = pCw.tile([128, D], F32, tag="ah")
                        nc.vector.tensor_scalar(out=ah, in0=xt, scalar1=mv[:, 0:1],
                                                scalar2=rstd, op0=ALU.subtract, op1=ALU.mult)
                        ptr = psTR.tile([128, 512], F32, tag="ptr")
                        for dd in range(4):
                            nc.tensor.matmul(ptr[:, 128 * dd:128 * dd + 128],
                                             ah[:, 128 * dd:128 * dd + 128], ident,
                                             is_transpose=True, start=(dd == 0),
                                             stop=(dd == 3), skip_group_check=True)
                        outv = aT_all.rearrange("p (g c) -> p g c", g=4)[:, :, 128 * t:128 * t + 128]
                        nc.scalar.copy(outv, ptr.rearrange("p (g c) -> p g c", g=4))

                # ============ stage 2+4 interleaved: V, then per f-tile pair qkv + 2 heads ============
                def emit_qkv_ft(ft):
                    for ch in range(2):
                        pq = psQK.tile([128, 384], F32, tag="pqv", name=f"pq_{ft}_{ch}")
                        for kt in range(4):
                            nc.tensor.matmul(pq, wqkv_sb[kt][:, 128 * ft:128 * ft + 128],
                                             aT[kt][:, 384 * ch:384 * ch + 384],
                                             start=(kt == 0),
                                             stop=(kt == 3 and not use_qkv_bias))
                        if use_qkv_bias:
                            nc.tensor.matmul(pq, wqkv_bias[:, 128 * ft:128 * ft + 128],
                                             ones_r[:, 384 * ch:384 * ch + 384],
                                             start=False, stop=True)
                        nc.vector.tensor_copy(qk_t[ft][:, 384 * ch:384 * ch + 384], pq)

                def emit_v():
                    for tt in range(6):
                        pv = psQK.tile([128, D], F32, tag="pqv", name=f"pv_{tt}")
                        for kt in range(4):
                            nc.tensor.matmul(pv, aT[kt][:, 128 * tt:128 * tt + 128],
                                             wqkv_sb[kt][:, 2 * D:3 * D],
                                             start=(kt == 0),
                                             stop=(kt == 3 and not use_qkv_bias))
                        if use_qkv_bias:
                            nc.tensor.matmul(pv, ones_r[:, 128 * tt:128 * tt + 128],
                                             wqkv_bias[:, 2 * D:3 * D], start=False, stop=True)
                        nc.vector.tensor_copy(v_sb[tt], pv)

                def emit_head(h):
                    hp = 64 * (h % 2)
                    pTraw = pD.tile([128, 6 * 256], BF16, tag="pTraw", name=f"pTraw{h}")
                    for jp in range(3):  # pairs (0,1) (2,3) (4,5)
                        pst = psST.tile([128, 512], F32, tag="pst", name=f"pst{h}_{jp}")
                        for jj in range(2):
                            j = 2 * jp + jj
                            nc.tensor.matmul(
                                pst[:, 256 * jj:256 * jj + 256],
                                qk_t[4 + h // 2][hp:hp + 64, 128 * j:128 * j + 128],
                                qk_t[h // 2][hp:hp + 64, _WJ[j]:_WJ[j] + 256],
                                start=(jj == 0), stop=(jj == 1),
                                skip_group_check=True)
                        nc.scalar.activation(out=pTraw[:, 512 * jp:512 * jp + 512],
                                             in_=pst, func=AF.Exp)
                    pT = pD.tile([128, 6 * 256], BF16, tag="pT", name=f"pT{h}")
                    nc.vector.tensor_mul(out=pT, in0=pTraw, in1=masks_sb)
                    tln = pD.tile([64, 768], F32, tag="tln", name=f"tln{h}")
                    trd = pD.tile([64, 768], F32, tag="trd", name=f"trd{h}")
                    pcxs = []
                    for c in range(3):
                        pcx = psCX.tile([64, 256], F32, tag="pcx", name=f"pcx{h}_{c}")
                        pdn = psDN.tile([64, 256], F32, tag="pdn", name=f"pdn{h}_{c}")
                        pcxs.append(pcx)
                        items = _CTX[c]
                        for idx, (j, lo, hi) in enumerate(items):
                            rhs = pT[:, 256 * j + lo - _WJ[j]:256 * j + hi - _WJ[j]]
                            first = idx == 0
                            last = idx == len(items) - 1
                            nc.tensor.matmul(pcx[:, lo - 256 * c:hi - 256 * c],
                                             v_sb[j][:, 64 * h:64 * h + 64], rhs,
                                             start=first, stop=last,
                                             skip_group_check=True)
                            nc.tensor.matmul(pdn[:, lo - 256 * c:hi - 256 * c],
                                             ones_bf64, rhs,
                                             start=first, stop=last,
                                             skip_group_check=True)
                        nc.scalar.activation(out=tln[:, 256 * c:256 * c + 256],
                                             in_=pdn, func=AF.Ln, bias=0.0, scale=1.0)
                    nc.scalar.activation(out=trd, in_=tln, func=AF.Exp,
                                         bias=0.0, scale=-1.0)
                    for c in range(3):
                        nc.vector.scalar_tensor_tensor(
                            out=ctxT[h][:, 256 * c:256 * c + 256],
                            in0=pcxs[c], scalar=1.0, in1=trd[:, 256 * c:256 * c + 256],
                            op0=ALU.mult, op1=ALU.mult)

                # ---- attention-phase + later loads ----
                nc.sync.dma_start(
                    out=masks_sb.rearrange("p (j q) -> p j q", j=6),
                    in_=d_masks.rearrange("(j p) q -> p j q", j=6))
                for h in range(H):
                    nc.sync.dma_start(out=wo_sb[h], in_=d_wo[64 * h:64 * h + 64, :])
                nc.sync.dma_start(out=wo_bias, in_=d_wo[D:D + 1, :])
                for t in range(5):
                    nc.sync.dma_start(out=x_q[t], in_=d_x[64 + 128 * t:192 + 128 * t, :])
                for i in range(4):
                    nc.sync.dma_start(out=convw_sb[i], in_=d_convw[128 * i:128 * i + 128, :])


                # ============ emission: V, then (qkv pair, 2 heads) x4 ============
                with tc.tile_pool(name="psQK", bufs=2, space="PSUM") as psQK, \
                     tc.tile_pool(name="pD", bufs=3) as pD, \
                     tc.tile_pool(name="psST", bufs=1, space="PSUM") as psST, \
                     tc.tile_pool(name="psCX", bufs=2, space="PSUM") as psCX, \
                     tc.tile_pool(name="psDN", bufs=2, space="PSUM") as psDN:
                    emit_v()
                    for pair in range(4):
                        emit_qkv_ft(pair)
                        emit_qkv_ft(4 + pair)
                        emit_head(2 * pair)
                        emit_head(2 * pair + 1)


            # ============ stage 5: w_o + residual ============
            with tc.tile_pool(name="psAT", bufs=3, space="PSUM") as psAT:
                for tt in range(5):
                    pat = psAT.tile([128, D], F32, tag="pat")
                    for h in range(H):
                        nc.tensor.matmul(pat, ctxT[h][:, 64 + 128 * tt:192 + 128 * tt],
                                         wo_sb[h], start=(h == 0),
                                         stop=(h == H - 1 and not use_wo_bias))
                    if use_wo_bias:
                        nc.tensor.matmul(pat, onesrow_bf, wo_bias, start=False, stop=True)
                    nc.vector.scalar_tensor_tensor(out=x1[tt], in0=pat, scalar=1.0,
                                                   in1=x_q[tt], op0=ALU.mult, op1=ALU.add)

        # pools pB/pC/pD exited above via stack; continue in fresh scope
        with tc.tile_pool(name="pE", bufs=1) as pE, \
             tc.tile_pool(name="pEw", bufs=2) as pEw, \
             tc.tile_pool(name="psT2", bufs=2, space="PSUM") as psT2, \
             tc.tile_pool(name="psPI", bufs=4, space="PSUM") as psPI, \
             tc.tile_pool(name="psPO", bufs=2, space="PSUM") as psPO:
            wpwi_sb = [pE.tile([128, FF], F32R, tag=f"wpwi{i}", name=f"wpwi{i}") for i in range(4)]
            wpwi_bias = pE.tile([1, FF], F32R)
            for i in range(4):
                nc.sync.dma_start(out=wpwi_sb[i], in_=d_wpwi[128 * i:128 * i + 128, :])
            nc.sync.dma_start(out=wpwi_bias, in_=d_wpwi[D:D + 1, :])
            wpwo_sb = [pE.tile([128, D], BF16, tag=f"wpwo{i}", name=f"wpwo{i}") for i in range(16)]
            wpwo_bias = pE.tile([1, D], BF16)
            for i in range(16):
                nc.scalar.dma_start(out=wpwo_sb[i], in_=d_wpwo[128 * i:128 * i + 128, :])
            nc.scalar.dma_start(out=wpwo_bias, in_=d_wpwo[FF:FF + 1, :])
            yT_all = pE.tile([128, 4 * NQ], F32, name="yT_all")
            yT = [yT_all[:, NQ * i:NQ * (i + 1)] for i in range(4)]
            convT = [pE.tile([128, CHUNK], F32R, tag=f"cT{i}", name=f"cT{i}") for i in range(4)]
            g_sb = [pE.tile([128, CHUNK], BF16, tag=f"g{i}", name=f"g{i}") for i in range(16)]
            x1s = [pE.tile([128, D], F32, tag=f"x1s{i}", name=f"x1s{i}") for i in range(4)]

            # x1s = x1 shifted by 64 rows (SBUF->SBUF DMA moves across partitions)
            for t4 in range(4):
                nc.sync.dma_start(out=x1s[t4][0:64, :], in_=x1[t4][64:128, :])
                nc.sync.dma_start(out=x1s[t4][64:128, :], in_=x1[t4 + 1][0:64, :])

            # ---- LN_f / LN_c ----
            epsq_sb = pE.tile([128, 1], F32, name="epsq_sb")
            nc.vector.memset(epsq_sb, EPS * EPS)
            for tt in range(5):
                st1 = pEw.tile([128, 6], F32, tag="st1")
                mv1 = pEw.tile([128, 2], F32, tag="mv1")
                nc.vector.bn_stats(out=st1, in_=x1[tt])
                nc.vector.bn_aggr(out=mv1, in_=st1)
                if not apply_gfbf:
                    # LN_c(LN_f(x)) with unit gain / zero bias collapses to a
                    # single normalization: (x - mu) / sqrt(v*(1+eps) + eps^2)
                    l2 = pEw.tile([128, 1], F32, tag="l2")
                    r2 = pEw.tile([128, 1], F32, tag="r2")
                    nc.scalar.activation(out=l2, in_=mv1[:, 1:2], func=AF.Ln,
                                         bias=epsq_sb, scale=1.0 + EPS)
                    nc.scalar.activation(out=r2, in_=l2, func=AF.Exp, bias=0.0, scale=-0.5)
                    n2 = pEw.tile([128, D], F32, tag="n2")
                    nc.vector.tensor_scalar(out=n2, in0=x1[tt], scalar1=mv1[:, 0:1],
                                            scalar2=r2, op0=ALU.subtract, op1=ALU.mult)
                else:
                    l1 = pEw.tile([128, 1], F32, tag="l1")
                    r1 = pEw.tile([128, 1], F32, tag="r1")
                    nc.scalar.activation(out=l1, in_=mv1[:, 1:2], func=AF.Ln,
                                         bias=eps_sb, scale=1.0)
                    nc.scalar.activation(out=r1, in_=l1, func=AF.Exp, bias=0.0, scale=-0.5)
                    n1 = pEw.tile([128, D], F32, tag="n1")
                    nc.vector.tensor_scalar(out=n1, in0=x1[tt], scalar1=mv1[:, 0:1],
                                            scalar2=r1, op0=ALU.subtract, op1=ALU.mult)
                    y1a = pEw.tile([128, D], F32, tag="y1a")
                    nc.vector.tensor_mul(out=y1a, in0=n1, in1=gf_sb)
                    nc.vector.tensor_add(out=n1, in0=y1a, in1=bf_sb)
                    st2 = pEw.tile([128, 6], F32, tag="st2")
                    mv2 = pEw.tile([128, 2], F32, tag="mv2")
                    nc.vector.bn_stats(out=st2, in_=n1)
                    nc.vector.bn_aggr(out=mv2, in_=st2)
                    l2 = pEw.tile([128, 1], F32, tag="l2")
                    r2 = pEw.tile([128, 1], F32, tag="r2")
                    nc.scalar.activation(out=l2, in_=mv2[:, 1:2], func=AF.Ln,
                                         bias=eps_sb, scale=1.0)
                    nc.scalar.activation(out=r2, in_=l2, func=AF.Exp, bias=0.0, scale=-0.5)
                    n2 = pEw.tile([128, D], F32, tag="n2")
                    nc.vector.tensor_scalar(out=n2, in0=n1, scalar1=mv2[:, 0:1],
                                            scalar2=r2, op0=ALU.subtract, op1=ALU.mult)
                if apply_gcbc:
                    y2a = pEw.tile([128, D], F32, tag="y2a")
                    nc.vector.tensor_mul(out=y2a, in0=n2, in1=gc_sb)
                    nc.vector.tensor_add(out=n2, in0=y2a, in1=bc_sb)
                    nc.vector.tensor_scalar_mul(out=n2, in0=n2, scalar1=ppad_sb[tt])
                pt2 = psT2.tile([128, 512], F32, tag="pt2")
                for dd in range(4):
                    nc.tensor.matmul(pt2[:, 128 * dd:128 * dd + 128],
                                     n2[:, 128 * dd:128 * dd + 128], ident,
                                     is_transpose=True, start=(dd == 0),
                                     stop=(dd == 3), skip_group_check=True)
                outv = yT_all.rearrange("p (g c) -> p g c", g=4)[:, :, 128 * tt:128 * tt + 128]
                nc.scalar.copy(outv, pt2.rearrange("p (g c) -> p g c", g=4))

            # ---- depthwise conv (feature-major, shifted adds) ----
            for dd in range(4):
                c1 = pEw.tile([128, CHUNK], F32, tag="c1")
                nc.vector.tensor_scalar_mul(out=c1, in0=yT[dd][:, 65:65 + CHUNK],
                                            scalar1=convw_sb[dd][:, 2:3])
                c2 = pEw.tile([128, CHUNK], F32, tag="c2")
                nc.vector.scalar_tensor_tensor(out=c2, in0=yT[dd][:, 63:63 + CHUNK],
                                               scalar=convw_sb[dd][:, 0:1], in1=c1,
                                               op0=ALU.mult, op1=ALU.add)
                nc.vector.scalar_tensor_tensor(out=convT[dd], in0=yT[dd][:, 64:64 + CHUNK],
                                               scalar=convw_sb[dd][:, 1:2], in1=c2,
                                               op0=ALU.mult, op1=ALU.add)

            # ---- pwi + gelu ----
            for ffi in range(16):
                ppi = psPI.tile([128, CHUNK], F32, tag="ppi")
                for kt in range(4):
                    nc.tensor.matmul(ppi, wpwi_sb[kt][:, 128 * ffi:128 * ffi + 128],
                                     convT[kt], start=(kt == 0),
                                     stop=(kt == 3 and not use_pwi_bias))
                if use_pwi_bias:
                    nc.tensor.matmul(ppi, wpwi_bias[:, 128 * ffi:128 * ffi + 128],
                                     ones_r[:, 0:CHUNK], start=False, stop=True)
                nc.scalar.activation(out=g_sb[ffi], in_=ppi, func=AF.Gelu)

            # ---- pwo + final residual ----
            for t4 in range(4):
                ppo = psPO.tile([128, D], F32, tag="ppo")
                for ffi in range(16):
                    nc.tensor.matmul(ppo, g_sb[ffi][:, 128 * t4:128 * t4 + 128],
                                     wpwo_sb[ffi], start=(ffi == 0),
                                     stop=(ffi == 15 and not use_pwo_bias))
                if use_pwo_bias:
                    nc.tensor.matmul(ppo, onesrow_bf, wpwo_bias, start=False, stop=True)
                nc.vector.scalar_tensor_tensor(out=out_sb[t4], in0=ppo, scalar=1.0,
                                               in1=x1s[t4], op0=ALU.mult, op1=ALU.add)
                if apply_outmask:
                    nc.vector.tensor_scalar_mul(out=out_sb[t4], in0=out_sb[t4],
                                                scalar1=om_sb[t4])
                nc.sync.dma_start(out=d_out[128 * t4:128 * t4 + 128, :], in_=out_sb[t4])

    _fix_excess_waits(nc)
    return nc



F8 = mybir.dt.float8e4
DRMODE = mybir.MatmulPerfMode.DoubleRow
WS = 32.0

def build_fp8_program():
    nc = bass.Bass(target_bir_lowering=False, trn_type="TRN2")

    d_x = nc.dram_tensor("x_ext", [EXT, D], F32, kind="ExternalInput")
    d_wqkv = nc.dram_tensor("wqkv8", [D, 3 * D], F8, kind="ExternalInput")
    d_wo = nc.dram_tensor("wo8", [D, D], F8, kind="ExternalInput")
    d_wpwi = nc.dram_tensor("wpwi8", [D, FF], F8, kind="ExternalInput")
    d_wpwo = nc.dram_tensor("wpwo8", [FF, D], F8, kind="ExternalInput")
    d_convw = nc.dram_tensor("convw32", [D, 3], F32, kind="ExternalInput")
    d_masks = nc.dram_tensor("masks", [6 * 128, 256], BF16, kind="ExternalInput")
    d_ident = nc.dram_tensor("identb", [128, 128], BF16, kind="ExternalInput")
    d_out = nc.dram_tensor("out", [CHUNK, D], F32, kind="ExternalOutput")

    ESC = 1.0 / (8.0 * WS * WS)     # exp scale: 1/sqrt(HD) / (32*32)
    GSC = 1.0 / (WS * WS)           # gelu input scale
    RS1 = 1.0 / (WS * WS)           # w_o residual scale
    RS2 = 1.0 / WS                  # pwo residual scale

    with tile.TileContext(nc) as tc, \
         tc.tile_pool(name="pA", bufs=1) as pA:
        # ---------------- persistent tiles ----------------
        identb = pA.tile([128, 128], BF16)
        onesb = pA.tile([128, 64], BF16)
        nc.vector.memset(onesb, 1.0)
        eps_sb = pA.tile([128, 1], F32)
        nc.vector.memset(eps_sb, EPS)
        epsq_sb = pA.tile([128, 1], F32)
        nc.vector.memset(epsq_sb, EPS * EPS)

        wqkv_all = pA.tile([128, 4 * 3 * D], F8, name="wqkv_all")
        wo_all = pA.tile([128, 4 * D], F8, name="wo_all")
        wpwi_all = pA.tile([128, 4 * FF], F8, name="wpwi_all")
        wpwo_all = pA.tile([128, 16 * D], F8, name="wpwo_all")
        convw_sb = pA.tile([128, 4 * 3], F32, name="convw_sb")
        masks_sb = pA.tile([128, 6 * 256], BF16, name="masks_sb")

        aT_all = pA.tile([128, 4 * EXT], F8, name="aT_all")
        qk_t = [pA.tile([128, EXT], F8, name=f"qk{i}") for i in range(8)]
        v_sb = [pA.tile([128, D], F8, name=f"v{i}") for i in range(6)]
        ctxT_all = pA.tile([128, 4 * EXT], F8, name="ctxT_all")
        x_q = [pA.tile([128, D], F32, name=f"xq{t}") for t in range(4)]
        x_qe = [pA.tile([1, D], F32, name=f"xqe{i}") for i in range(2)]
        x1 = [pA.tile([128, D], F32, name=f"x1_{t}") for t in range(4)]
        x1e = [pA.tile([1, D], F32, name=f"x1e{i}") for i in range(2)]
        yT_all = pA.tile([128, 4 * 514], BF16, name="yT_all")
        convT = pA.tile([128, 4 * CHUNK], F8, name="convT")
        g_all = pA.tile([128, 16 * CHUNK], F8, name="g_all")
        out_sb = [pA.tile([128, D], F32, name=f"out{t}") for t in range(4)]
        mv_all = pA.tile([128, 2 * 6], F32, name="mv_all")
        rstd_a = pA.tile([128, 6], F32, name="rstd_a")
        mvf_all = pA.tile([128, 2 * 4], F32, name="mvf_all")
        rstd_f = pA.tile([128, 4], F32, name="rstd_f")
        mv_e = [pA.tile([1, 2], F32, name=f"mv_e{i}") for i in range(2)]
        rstd_e = [pA.tile([1, 1], F32, name=f"rstd_e{i}") for i in range(2)]

        wqkv4 = wqkv_all.rearrange("p (g c) -> p g c", g=4)
        wo4 = wo_all.rearrange("p (g c) -> p g c", g=4)
        wpwi4 = wpwi_all.rearrange("p (g c) -> p g c", g=4)
        wpwo16 = wpwo_all.rearrange("p (g c) -> p g c", g=16)
        aT4 = aT_all.rearrange("p (g c) -> p g c", g=4)
        ctxT4 = ctxT_all.rearrange("p (g c) -> p g c", g=4)
        convT4 = convT.rearrange("p (g c) -> p g c", g=4)
        g16 = g_all.rearrange("p (g c) -> p g c", g=16)
        yT4 = yT_all.rearrange("p (g c) -> p g c", g=4)
        cw4 = convw_sb.rearrange("p (g c) -> p g c", g=4)

        # ---------------- all input DMAs up front ----------------
        # sync queue gets ident first (first transpose needs it); x chunks
        # follow in phase 1. scalar queue: wo, x_q, edge rows, wpwi/wpwo.
        nc.sync.dma_start(out=identb, in_=d_ident[:, :])

        # ---------------- phase 1: x load, LN_a, transpose, V ----------------
        with tc.tile_pool(name="pCw", bufs=3) as pCw, \
             tc.tile_pool(name="psTR", bufs=2, space="PSUM") as psTR, \
             tc.tile_pool(name="psV", bufs=2, space="PSUM") as psV:
            xts = [pA.tile([128, D], F32, name=f"xt_{t}") for t in range(6)]
            # x tiles pipelined on alternating queues for fast first-tile
            for t in range(6):
                eng = nc.sync if t % 2 == 0 else nc.scalar
                eng.dma_start(out=xts[t], in_=d_x[128 * t:128 * t + 128, :])
            nc.sync.dma_start(out=wqkv_all.rearrange("p (g c) -> p g c", g=4),
                              in_=d_wqkv.rearrange("(g p) c -> p g c", g=4))
            nc.sync.dma_start(
                out=masks_sb.rearrange("p (j q) -> p j q", j=6),
                in_=d_masks.rearrange("(j p) q -> p j q", j=6))
            # later-phase loads on the idle sync queue, behind x/wqkv/masks
            nc.sync.dma_start(out=wo_all.rearrange("p (g c) -> p g c", g=4),
                              in_=d_wo.rearrange("(g p) c -> p g c", g=4))
            for t_ in range(4):
                nc.sync.dma_start(out=x_q[t_],
                                  in_=d_x[128 + 128 * t_:256 + 128 * t_, :])
            nc.sync.dma_start(out=x_qe[0], in_=d_x[127:128, :])
            nc.sync.dma_start(out=x_qe[1], in_=d_x[640:641, :])
            nc.sync.dma_start(out=convw_sb.rearrange("p (g c) -> p g c", g=4),
                              in_=d_convw.rearrange("(g p) c -> p g c", g=4))
            nc.sync.dma_start(out=wpwi_all.rearrange("p (g c) -> p g c", g=4),
                              in_=d_wpwi.rearrange("(g p) c -> p g c", g=4))
            nc.sync.dma_start(out=wpwo_all.rearrange("p (g c) -> p g c", g=16),
                              in_=d_wpwo.rearrange("(g p) c -> p g c", g=16))

            def emit_v(tt):
                pv = psV.tile([128, D], F32, tag="pv", name=f"pv{tt}")
                for pj in range(2):
                    nc.tensor.matmul(
                        pv,
                        aT4[:, 2 * pj:2 * pj + 2, 128 * tt:128 * tt + 128],
                        wqkv4[:, 2 * pj:2 * pj + 2, 2 * D:3 * D],
                        start=(pj == 0), stop=(pj == 1), perf_mode=DRMODE)
                nc.vector.tensor_copy(v_sb[tt], pv)

            for t in range(6):
                xt = xts[t]
                st = pCw.tile([128, 6], F32, tag="st")
                nc.vector.bn_stats(out=st, in_=xt)
                nc.vector.bn_aggr(out=mv_all[:, 2 * t:2 * t + 2], in_=st)
                if t % 2 == 1:
                    # batched rstd for tiles t-1, t
                    vsl = mv_all.rearrange("p (t two) -> p two t", two=2)[:, 1:2, t - 1:t + 1]
                    lsl = pCw.tile([128, 2], F32, tag="lv")
                    nc.scalar.activation(out=lsl, in_=vsl, func=AF.Ln,
                                         bias=eps_sb, scale=1.0)
                    nc.scalar.activation(out=rstd_a[:, t - 1:t + 1], in_=lsl,
                                         func=AF.Exp, bias=0.0, scale=-0.5)
                for tt in (t - 1, t) if t % 2 == 1 else ():
                    xtt = xts[tt]
                    ah = pCw.tile([128, D], BF16, tag="ah")
                    nc.gpsimd.tensor_scalar(
                        out=ah, in0=xtt,
                        scalar1=mv_all[:, 2 * tt:2 * tt + 1],
                        scalar2=rstd_a[:, tt:tt + 1],
                        op0=ALU.subtract, op1=ALU.mult)
                    ptr = psTR.tile([128, 1024], BF16, tag="ptr")
                    for dd in range(4):
                        nc.tensor.matmul(ptr[:, 128 * dd:128 * dd + 128],
                                         ah[:, 128 * dd:128 * dd + 128], identb,
                                         is_transpose=True, start=(dd == 0),
                                         stop=(dd == 3), skip_group_check=True)
                    nc.scalar.copy(aT4[:, :, 128 * tt:128 * tt + 128],
                                   ptr[:, 0:512].rearrange("p (g c) -> p g c", g=4))

            for tt in range(6):
                emit_v(tt)

        # ---------------- phase 2+3: QKV f-major + heads ----------------
        with tc.tile_pool(name="pD", bufs=2) as pD:

            def emit_qk_ft(psQK, ft):
                pq = psQK.tile([128, EXT], F32, tag="pq", name=f"pq{ft}")
                for c0, c1 in ((0, 512), (512, 768)):
                    for pj in range(2):
                        nc.tensor.matmul(
                            pq[:, c0:c1],
                            wqkv4[:, 2 * pj:2 * pj + 2, 128 * ft:128 * ft + 128],
                            aT4[:, 2 * pj:2 * pj + 2, c0:c1],
                            start=(pj == 0), stop=(pj == 1),
                            perf_mode=DRMODE, skip_group_check=True)
                nc.scalar.copy(qk_t[ft], pq)

            def emit_head(psST, psCX, h):
                hp = 64 * (h % 2)
                qt = qk_t[h // 2]
                kt_ = qk_t[4 + h // 2]
                pTraw = pD.tile([128, 6 * 256], BF16, tag="pTraw", name=f"pTraw{h}")
                for g in range(2):
                    pst = psST.tile([128, 1024], F32, tag="pst", name=f"pst{h}_{g}")
                    for jj in range(3):
                        j = 3 * g + jj
                        nc.tensor.matmul(
                            pst[:, 256 * jj:256 * jj + 256],
                            kt_[hp:hp + 64, 128 * j:128 * j + 128],
                            qt[hp:hp + 64, _WJ[j]:_WJ[j] + 256],
                            start=(jj % 2 == 0), stop=(jj % 2 == 1),
                            skip_group_check=True)
                    nc.scalar.activation(out=pTraw[:, 768 * g:768 * g + 768],
                                         in_=pst[:, 0:768], func=AF.Exp,
                                         bias=0.0, scale=ESC)
                pT = pD.tile([128, 6 * 256], BF16, tag="pT", name=f"pT{h}")
                nc.vector.tensor_mul(out=pT, in0=pTraw, in1=masks_sb)
                pcxdn = psCX.tile([128, 1024], F32, tag="pcxdn", name=f"pcxdn{h}")
                for c in range(3):
                    items = _CTX[c]
                    for idx, (j, lo, hi) in enumerate(items):
                        rhs = pT[:, 256 * j + lo - _WJ[j]:256 * j + hi - _WJ[j]]
                        first = idx == 0 and c in (0, 2)
                        last = idx == len(items) - 1
                        nc.tensor.matmul(pcxdn[0:64, lo:hi],
                                         v_sb[j][:, 64 * h:64 * h + 64], rhs,
                                         start=first, stop=last,
                                         skip_group_check=True)
                        nc.tensor.matmul(pcxdn[64:128, lo:hi],
                                         onesb, rhs,
                                         start=first, stop=last,
                                         skip_group_check=True)
                trd = pD.tile([64, EXT], F32, tag="trd", name=f"trd{h}")
                if h % 2 == 0:
                    nc.vector.reciprocal(out=trd, in_=pcxdn[64:128, 0:EXT])
                else:
                    tln = pD.tile([64, EXT], F32, tag="tln", name=f"tln{h}")
                    nc.scalar.activation(out=tln, in_=pcxdn[64:128, 0:EXT],
                                         func=AF.Ln, bias=0.0, scale=1.0)
                    nc.scalar.activation(out=trd, in_=tln, func=AF.Exp,
                                         bias=0.0, scale=-1.0)
                nc.vector.tensor_tensor(
                    out=ctxT4[hp:hp + 64, h // 2, :],
                    in0=pcxdn[0:64, 0:EXT], in1=trd, op=ALU.mult)

            with tc.tile_pool(name="psQK", bufs=2, space="PSUM") as psQK:
                for ft in (0, 4, 1, 5, 2, 6, 3, 7):
                    emit_qk_ft(psQK, ft)
            with tc.tile_pool(name="psST", bufs=2, space="PSUM") as psST, \
                 tc.tile_pool(name="psCX", bufs=2, space="PSUM") as psCX:
                for h in range(H):
                    emit_head(psST, psCX, h)

        # ---------------- phase 4: w_o + residual + LN_f + transpose ----------
        with tc.tile_pool(name="pEw", bufs=3) as pEw, \
             tc.tile_pool(name="psAT", bufs=2, space="PSUM") as psAT, \
             tc.tile_pool(name="psAE", bufs=1, space="PSUM") as psAE, \
             tc.tile_pool(name="psT2", bufs=2, space="PSUM") as psT2:
            for tt in range(4):
                pat = psAT.tile([128, D], F32, tag="pat", name=f"pat{tt}")
                for g in range(2):
                    nc.tensor.matmul(
                        pat,
                        ctxT4[:, 2 * g:2 * g + 2, 128 + 128 * tt:256 + 128 * tt],
                        wo4[:, 2 * g:2 * g + 2, :],
                        start=(g == 0), stop=(g == 1), perf_mode=DRMODE)
                nc.vector.scalar_tensor_tensor(
                    out=x1[tt], in0=pat, scalar=RS1, in1=x_q[tt],
                    op0=ALU.mult, op1=ALU.add)
                stf = pEw.tile([128, 6], F32, tag="stf")
                nc.vector.bn_stats(out=stf, in_=x1[tt])
                nc.vector.bn_aggr(out=mvf_all[:, 2 * tt:2 * tt + 2], in_=stf)
            # edge rows (ext 127 and 640) -> two [1,512] psum tiles
            pes = []
            for ei, col in enumerate((127, 640)):
                pe_ = psAE.tile([1, D], F32, tag=f"pate{ei}", name=f"pate{ei}")
                for g in range(2):
                    nc.tensor.matmul(
                        pe_,
                        ctxT4[:, 2 * g:2 * g + 2, col:col + 1],
                        wo4[:, 2 * g:2 * g + 2, :],
                        start=(g == 0), stop=(g == 1), perf_mode=DRMODE)
                pes.append(pe_)
            for ei in range(2):
                nc.vector.scalar_tensor_tensor(
                    out=x1e[ei], in0=pes[ei], scalar=RS1,
                    in1=x_qe[ei], op0=ALU.mult, op1=ALU.add)
                ste = pEw.tile([1, 6], F32, tag=f"ste{ei}")
                nc.vector.bn_stats(out=ste, in_=x1e[ei])
                nc.vector.bn_aggr(out=mv_e[ei], in_=ste)

            # batched LN_f scalars (collapsed double-LN)
            vslf = mvf_all.rearrange("p (t two) -> p two t", two=2)[:, 1:2, :]
            lf = pEw.tile([128, 4], F32, tag="lf")
            nc.scalar.activation(out=lf, in_=vslf, func=AF.Ln,
                                 bias=epsq_sb, scale=1.0 + EPS)
            nc.scalar.activation(out=rstd_f, in_=lf, func=AF.Exp,
                                 bias=0.0, scale=-0.5)
            for ei in range(2):
                le = pEw.tile([1, 1], F32, tag=f"le{ei}")
                nc.scalar.activation(out=le, in_=mv_e[ei][:, 1:2], func=AF.Ln,
                                     bias=epsq_sb[0:1, :], scale=1.0 + EPS)
                nc.scalar.activation(out=rstd_e[ei], in_=le, func=AF.Exp,
                                     bias=0.0, scale=-0.5)

            for tt in range(4):
                n2 = pEw.tile([128, D], BF16, tag="n2")
                nc.gpsimd.tensor_scalar(
                    out=n2, in0=x1[tt],
                    scalar1=mvf_all[:, 2 * tt:2 * tt + 1],
                    scalar2=rstd_f[:, tt:tt + 1],
                    op0=ALU.subtract, op1=ALU.mult)
                pt2 = psT2.tile([128, 1024], BF16, tag="pt2")
                for dd in range(4):
                    nc.tensor.matmul(pt2[:, 128 * dd:128 * dd + 128],
                                     n2[:, 128 * dd:128 * dd + 128], identb,
                                     is_transpose=True, start=(dd == 0),
                                     stop=(dd == 3), skip_group_check=True)
                nc.scalar.copy(yT4[:, :, 1 + 128 * tt:129 + 128 * tt],
                               pt2[:, 0:512].rearrange("p (g c) -> p g c", g=4))
            # edge LN + transpose -> yT cols 0 and 513
            pt2e = psAE.tile([128, 1024], BF16, tag="pt2e", name="pt2e")
            for ei in range(2):
                n2e = pEw.tile([1, D], BF16, tag=f"n2e{ei}")
                nc.gpsimd.tensor_scalar(
                    out=n2e, in0=x1e[ei], scalar1=mv_e[ei][:, 0:1],
                    scalar2=rstd_e[ei], op0=ALU.subtract, op1=ALU.mult)
                for dd in range(4):
                    k = 2 * (4 * ei + dd)
                    nc.tensor.matmul(pt2e[:, k:k + 1],
                                     n2e[:, 128 * dd:128 * dd + 128],
                                     identb[0:1, 0:1],
                                     is_transpose=True,
                                     start=(ei == 0 and dd == 0),
                                     stop=(ei == 1 and dd == 3),
                                     skip_group_check=True)
            ecol = (0, 513)
            for ei in range(2):
                for dd in range(4):
                    k = 2 * (4 * ei + dd)
                    nc.scalar.copy(yT4[:, dd, ecol[ei]:ecol[ei] + 1],
                                   pt2e[:, k:k + 1])

        # ---------------- phase 5: conv ----------------
        with tc.tile_pool(name="pF", bufs=2) as pF:
            for dd in range(4):
                c1 = pF.tile([128, CHUNK], BF16, tag="c1")
                nc.vector.tensor_scalar_mul(out=c1, in0=yT4[:, dd, 2:514],
                                            scalar1=cw4[:, dd, 2:3])
                c2 = pF.tile([128, CHUNK], BF16, tag="c2")
                nc.vector.scalar_tensor_tensor(
                    out=c2, in0=yT4[:, dd, 0:512], scalar=cw4[:, dd, 0:1],
                    in1=c1, op0=ALU.mult, op1=ALU.add)
                nc.vector.scalar_tensor_tensor(
                    out=convT4[:, dd, :], in0=yT4[:, dd, 1:513],
                    scalar=cw4[:, dd, 1:2], in1=c2, op0=ALU.mult, op1=ALU.add)

        # ---------------- phase 6: pwi + gelu ----------------
        with tc.tile_pool(name="psPI", bufs=2, space="PSUM") as psPI, \
             tc.tile_pool(name="psPO", bufs=1, space="PSUM") as psPO:
            ppos = [psPO.tile([128, D], F32, tag=f"ppo{t4}", name=f"ppo{t4}")
                    for t4 in range(4)]
            for j in range(8):
                ppi = psPI.tile([128, 2 * CHUNK], F32, tag="ppi", name=f"ppi{j}")
                for sub in range(2):
                    ffi = 2 * j + sub
                    for pj in range(2):
                        nc.tensor.matmul(
                            ppi[:, 512 * sub:512 * sub + 512],
                            wpwi4[:, 2 * pj:2 * pj + 2, 128 * ffi:128 * ffi + 128],
                            convT4[:, 2 * pj:2 * pj + 2, :],
                            start=(pj == 0), stop=(pj == 1),
                            perf_mode=DRMODE, skip_group_check=True)
                nc.scalar.activation(
                    out=g16[:, 2 * j:2 * j + 2, :], in_=ppi,
                    func=AF.Gelu, bias=0.0, scale=GSC)
                for t4 in range(4):
                    nc.tensor.matmul(
                        ppos[t4],
                        g16[:, 2 * j:2 * j + 2, 128 * t4:128 * t4 + 128],
                        wpwo16[:, 2 * j:2 * j + 2, :],
                        start=(j == 0), stop=(j == 7), perf_mode=DRMODE,
                        skip_group_check=True)
            for t4 in range(4):
                nc.vector.scalar_tensor_tensor(
                    out=out_sb[t4], in0=ppos[t4], scalar=RS2, in1=x1[t4],
                    op0=ALU.mult, op1=ALU.add)
                nc.sync.dma_start(out=d_out[128 * t4:128 * t4 + 128, :],
                                  in_=out_sb[t4])

    return nc


_PROG_CACHE = {}


def _get_program(cfg):
    if cfg not in _PROG_CACHE:
        if cfg == "fp8":
            nc = build_fp8_program()
            _fix_excess_waits(nc)
            _PROG_CACHE[cfg] = nc
        else:
            _PROG_CACHE[cfg] = _build_program(cfg)
    return _PROG_CACHE[cfg]


def _build_masks(key_mask_row, start):
    """Multiplicative {0,1} masks [6*128, 256] bf16 for one core (k-major S^T)."""
    out = np.zeros((6, 128, 256), np.float32)
    # key usability per ext position
    g_all = start - 128 + np.arange(EXT)
    k_ok = (g_all >= 0) & (g_all < S)
    k_ok &= key_mask_row[np.clip(g_all, 0, S - 1)]
    # a query is "live" if it is a real query position AND has >=1 usable
    # in-band key; otherwise it self-attends (finite junk, later zeroed --
    # matches the reference, whose all-masked rows are zeroed by the final
    # mask multiply before anything can observe them)
    q_live = np.zeros(EXT, bool)
    for e_q in range(64, 704):
        g_q = start - 128 + e_q
        if 0 <= g_q < S:
            lo, hi = max(0, e_q - WIN), min(EXT, e_q + WIN + 1)
            q_live[e_q] = k_ok[lo:hi].any()
    for j in range(6):
        kl = np.arange(128)
        ql = np.arange(256)
        e_k = 128 * j + kl[:, None]           # [128, 1]
        e_q = _WJ[j] + ql[None, :]            # [1, 256]
        band = np.abs(e_q - e_k) <= WIN
        ok = (q_live[e_q] & k_ok[e_k] & band) | ((~q_live[e_q]) & (e_k == e_q))
        out[j][np.broadcast_to(ok, (128, 256))] = 1.0
    return np.ascontiguousarray(out.reshape(6 * 128, 256).astype(ml_dtypes.bfloat16))


def prepare(**inputs):
    x = np.ascontiguousarray(np.asarray(inputs["x"], np.float32))
    key_mask = np.asarray(inputs["mask"]).astype(bool)
    ln_a_g = np.asarray(inputs["ln_a_g"], np.float32)
    ln_a_b = np.asarray(inputs["ln_a_b"], np.float32)
    w_qkv = np.asarray(inputs["w_qkv"], np.float32)
    b_qkv = np.asarray(inputs["b_qkv"], np.float32)
    w_o = np.asarray(inputs["w_o"], np.float32)
    b_o = np.asarray(inputs["b_o"], np.float32)
    ln_f_g = np.asarray(inputs["ln_f_g"], np.float32)
    ln_f_b = np.asarray(inputs["ln_f_b"], np.float32)
    ln_c_g = np.asarray(inputs["ln_c_g"], np.float32)
    ln_c_b = np.asarray(inputs["ln_c_b"], np.float32)
    dw_w = np.asarray(inputs["dw_w"], np.float32)
    dw_b = np.asarray(inputs["dw_b"], np.float32)
    pwi_w = np.asarray(inputs["pwi_w"], np.float32)
    pwi_b = np.asarray(inputs["pwi_b"], np.float32)
    pwo_w = np.asarray(inputs["pwo_w"], np.float32)
    pwo_b = np.asarray(inputs["pwo_b"], np.float32)

    apply_gfbf = not (np.all(ln_f_g == 1.0) and np.all(ln_f_b == 0.0))
    apply_gcbc = not np.all(ln_c_b == 0.0)
    apply_outmask = not key_mask.all()

    # ---- host weight prep ----
    scale = np.float32(1.0 / np.sqrt(HD))
    Wt = w_qkv.T.astype(np.float64)                      # [D, 3D]
    Wg = Wt * ln_a_g[:, None].astype(np.float64)
    brow = ln_a_b.astype(np.float64) @ Wt + b_qkv
    Wg[:, :D] *= scale
    brow[:D] *= scale
    wqkv_hat = np.ascontiguousarray(
        np.vstack([Wg, brow[None, :]]).astype(np.float32))

    wo_hat = np.ascontiguousarray(
        np.vstack([w_o.T, b_o[None, :]]).astype(ml_dtypes.bfloat16))

    if not apply_gcbc:
        convw_eff = dw_w[:, 0, :] * ln_c_g[:, None]
        cc = dw_b
    else:
        convw_eff = dw_w[:, 0, :]
        cc = dw_b
    convw_eff = np.ascontiguousarray(convw_eff.astype(np.float32))

    pwi_bias_row = pwi_b + pwi_w @ cc
    wpwi_hat = np.ascontiguousarray(
        np.vstack([pwi_w.T, pwi_bias_row[None, :]]).astype(np.float32))
    wpwo_hat = np.ascontiguousarray(
        np.vstack([pwo_w.T, pwo_b[None, :]]).astype(ml_dtypes.bfloat16))
    use_qkv_bias = bool(np.any(brow != 0.0))
    use_wo_bias = bool(np.any(b_o != 0.0))
    use_pwi_bias = bool(np.any(pwi_bias_row != 0.0))
    use_pwo_bias = bool(np.any(pwo_b != 0.0))
    cfg = (apply_gfbf, apply_gcbc, apply_outmask,
           use_qkv_bias, use_wo_bias, use_pwi_bias, use_pwo_bias)

    ident = np.eye(128, dtype=np.float32)
    ones_r = np.ones((1, EXT), np.float32)

    fast = not any(cfg)
    if fast:
        E4 = ml_dtypes.float8_e4m3
        wqkv8 = np.ascontiguousarray(
            (w_qkv.T * ln_a_g[:, None] * WS).astype(E4))
        wo8 = np.ascontiguousarray((w_o.T * WS).astype(E4))
        wpwi8 = np.ascontiguousarray((pwi_w.T * WS).astype(E4))
        wpwo8 = np.ascontiguousarray((pwo_w.T * WS).astype(E4))
        convw32 = np.ascontiguousarray(
            (dw_w[:, 0, :] * ln_c_g[:, None] * WS).astype(np.float32))
        identb = np.eye(128, dtype=ml_dtypes.bfloat16)
        nc = _get_program("fp8")
        in_maps = []
        for core in range(NCORES):
            b, c = divmod(core, 4)
            start = CHUNK * c
            lo, hi = start - 128, start + 640
            x_ext = np.zeros((EXT, D), np.float32)
            s0, s1 = max(lo, 0), min(hi, S)
            x_ext[s0 - lo:s1 - lo] = x[b, s0:s1]
            in_maps.append({
                "x_ext": x_ext,
                "wqkv8": wqkv8,
                "wo8": wo8,
                "wpwi8": wpwi8,
                "wpwo8": wpwo8,
                "convw32": convw32,
                "masks": _build_masks(key_mask[b], start),
                "identb": identb,
            })

        def assemble(per_core_outs):
            out = np.empty((B, S, D), np.float32)
            for core in range(NCORES):
                b, c = divmod(core, 4)
                out[b, CHUNK * c:CHUNK * (c + 1)] = per_core_outs[core]
            return out

        return nc, in_maps, assemble

    nc = _get_program(cfg)

    in_maps = []
    for core in range(NCORES):
        b, c = divmod(core, 4)
        start = CHUNK * c
        lo, hi = start - 128, start + 640
        x_ext = np.zeros((EXT, D), np.float32)
        s0, s1 = max(lo, 0), min(hi, S)
        x_ext[s0 - lo:s1 - lo] = x[b, s0:s1]
        m = {
            "x_ext": x_ext,
            "wqkv": wqkv_hat,
            "wo": wo_hat,
            "wpwi": wpwi_hat,
            "wpwo": wpwo_hat,
            "convw": convw_eff,
            "masks": _build_masks(key_mask[b], start),
            "ident": ident,
            "ones_r": ones_r,
        }
        if apply_gfbf:
            m["gf_b"] = np.ascontiguousarray(
                np.broadcast_to(ln_f_g[None, :], (128, D)).astype(np.float32))
            m["bf_b"] = np.ascontiguousarray(
                np.broadcast_to(ln_f_b[None, :], (128, D)).astype(np.float32))
        if apply_gcbc:
            m["gc_b"] = np.ascontiguousarray(
                np.broadcast_to(ln_c_g[None, :], (128, D)).astype(np.float32))
            m["bc_b"] = np.ascontiguousarray(
                np.broadcast_to(ln_c_b[None, :], (128, D)).astype(np.float32))
            gq = start - 64 + np.arange(NQ)
            m["ppad"] = ((gq >= 0) & (gq < S)).astype(np.float32)[:, None]
        if apply_outmask:
            m["outmask"] = key_mask[b, start:start + CHUNK].astype(np.float32)[:, None]
        in_maps.append(m)

    def assemble(per_core_outs):
        out = np.empty((B, S, D), np.float32)
        for core in range(NCORES):
            b, c = divmod(core, 4)
            out[b, CHUNK * c:CHUNK * (c + 1)] = per_core_outs[core]
        return out

    return nc, in_maps, assemble


def kernel(**inputs):
    nc, in_maps, assemble = prepare(**inputs)
    res = run_bass_kernel_spmd(nc, in_maps, core_ids=list(range(NCORES)))
    return assemble([res.results[core]["out"] for core in range(NCORES)])

